# revision 1
# baseline (speedup 1.0000x reference)
"""Trainium2 Bass kernel for nn_KITRO (gnn_message_passing).

Pure data parallel over 8 NeuronCores: batch 8192 -> 1024 per core.
Per core:
  Phase A (depth MLP, transposed dataflow, bf16 matmuls / fp32 accum):
    features f32 --cast DMA--> bf16 DRAM scratch --xbar transpose DMA-->
    XT [512f, rows] in SBUF; h1T = relu(W1^T XT + b1); h2T = relu(W2^T h1T + b2);
    depths = W3^T h2T + b3 -> DRAM scratch.
    Also zT = (0.5*cW1[4:])^T XT (bone-feature projection, averaged later in
    32-dim z space instead of 512-dim feature space -- algebraically exact).
  Phase B (3 bone refinement iterations):
    batch-on-partition geometry (bone vectors / length / direction) on DVE/ACT,
    tiny bone MLP via block-diagonal matmuls in transposed layout on PE,
    per-batch mean over bones fused into the last matmul accumulation.
"""

import sys

if "/opt/trn_rl_repo" not in sys.path:
    sys.path.insert(0, "/opt/trn_rl_repo")

import numpy as np

import concourse.bass as bass
import concourse.mybir as mybir
import concourse.tile as tile
from concourse import bacc
from concourse.bass_utils import run_bass_kernel_spmd
from concourse.masks import make_identity

F32 = mybir.dt.float32
BF16 = mybir.dt.bfloat16
AF = mybir.ActivationFunctionType
OP = mybir.AluOpType

NCORE = 8
B = 8192
BC = B // NCORE          # 1024 batches per core
J = 25
FD = 512
HD = 1024
ROWS = BC * J            # 25600 rows per core
RC = 512                 # row-chunk
NCH = ROWS // RC         # 50 chunks
EPS = 1e-8

_CACHE = {}


def _build_nc(stage="full"):
    nc = bacc.Bacc("TRN2", target_bir_lowering=False, debug=False,
                   num_devices=NCORE)

    # ---- per-core DRAM I/O ----
    feats = nc.dram_tensor("features", [BC, J, FD], F32, kind="ExternalInput")
    p2d = nc.dram_tensor("poses_2d", [BC, J, 2], F32, kind="ExternalInput")
    conf = nc.dram_tensor("confidence", [BC, J], F32, kind="ExternalInput")
    dW1 = nc.dram_tensor("dW1", [FD, HD], F32, kind="ExternalInput")
    db1 = nc.dram_tensor("db1", [HD], F32, kind="ExternalInput")
    dW2 = nc.dram_tensor("dW2", [HD, FD], F32, kind="ExternalInput")
    db2 = nc.dram_tensor("db2", [FD], F32, kind="ExternalInput")
    dW3 = nc.dram_tensor("dW3", [FD, 1], F32, kind="ExternalInput")
    db3 = nc.dram_tensor("db3", [1], F32, kind="ExternalInput")
    cW1 = nc.dram_tensor("cW1", [4 + FD, 32], F32, kind="ExternalInput")
    cb1 = nc.dram_tensor("cb1", [32], F32, kind="ExternalInput")
    cW2 = nc.dram_tensor("cW2", [32, 64], F32, kind="ExternalInput")
    cb2 = nc.dram_tensor("cb2", [64], F32, kind="ExternalInput")
    cW3 = nc.dram_tensor("cW3", [64, 3], F32, kind="ExternalInput")
    cb3 = nc.dram_tensor("cb3", [3], F32, kind="ExternalInput")
    out = nc.dram_tensor("out", [BC, J, 3], F32, kind="ExternalOutput")

    (feats, p2d, conf, dW1, db1, dW2, db2, dW3, db3,
     cW1, cb1, cW2, cb2, cW3, cb3, out) = (
        t.ap() for t in (feats, p2d, conf, dW1, db1, dW2, db2, dW3, db3,
                         cW1, cb1, cW2, cb2, cW3, cb3, out))

    feats_flat = feats.flatten_outer_dims()          # [ROWS, FD]

    with tile.TileContext(nc) as tc:
        import contextlib
        with contextlib.ExitStack() as ctx:
            const = ctx.enter_context(tc.tile_pool(name="const", bufs=1))
            dram = ctx.enter_context(
                tc.tile_pool(name="dram", bufs=1, space="DRAM"))

            # ---- constants / weights (bf16 via casting SWDGE DMA) ----
            id_bf = const.tile([128, 128], BF16, tag="id")
            make_identity(nc, id_bf)
            id3 = const.tile([3, 3], F32, tag="id3")
            make_identity(nc, id3)

            w1 = const.tile([128, 4, HD], BF16, tag="w1")
            nc.gpsimd.dma_start(
                out=w1, in_=dW1.rearrange("(kh p) m -> p kh m", p=128))
            w1z = const.tile([128, 4, 32], BF16, tag="w1z")
            nc.gpsimd.dma_start(
                out=w1z, in_=cW1[4:].rearrange("(kh p) m -> p kh m", p=128))
            # fold the 0.5 bone-average factor into the z-projection weights
            nc.vector.tensor_scalar_mul(w1z, w1z, 0.5)
            w2 = const.tile([128, 8, FD], BF16, tag="w2")
            nc.gpsimd.dma_start(
                out=w2, in_=dW2.rearrange("(kh p) n -> p kh n", p=128))
            w3 = const.tile([128, 4, 1], BF16, tag="w3")
            nc.gpsimd.dma_start(
                out=w3, in_=dW3.rearrange("(kh p) o -> p kh o", p=128))

            # block-diagonal bone weights
            blkW1 = const.tile([64, 512], BF16, tag="blkW1")   # 16x cW1[:4]
            nc.vector.memset(blkW1, 0.0)
            for d in range(16):
                nc.gpsimd.dma_start(
                    out=blkW1[4 * d:4 * d + 4, 32 * d:32 * d + 32],
                    in_=cW1[0:4, :])
            blkW2 = const.tile([128, 256], BF16, tag="blkW2")  # 4x cW2
            nc.vector.memset(blkW2, 0.0)
            for d in range(4):
                nc.gpsimd.dma_start(
                    out=blkW2[32 * d:32 * d + 32, 64 * d:64 * d + 64],
                    in_=cW2[:, :])
            w3stk = const.tile([128, 3], BF16, tag="w3stk")    # cW3 stacked 2x
            for d in range(2):
                nc.gpsimd.dma_start(out=w3stk[64 * d:64 * d + 64, :],
                                    in_=cW3[:, :])

            # biases
            db1_sb = const.tile([128, 8], F32, tag="db1")
            nc.sync.dma_start(out=db1_sb,
                              in_=db1.rearrange("(m p) -> p m", p=128))
            db2_sb = const.tile([128, 4], F32, tag="db2")
            nc.sync.dma_start(out=db2_sb,
                              in_=db2.rearrange("(m p) -> p m", p=128))
            db3_sb = const.tile([1, 1], F32, tag="db3")
            nc.sync.dma_start(out=db3_sb,
                              in_=db3.rearrange("(a o) -> a o", a=1))
            cb1_sb = const.tile([128, 1], F32, tag="cb1")
            for q in range(4):
                nc.sync.dma_start(out=cb1_sb[32 * q:32 * q + 32, :],
                                  in_=cb1.rearrange("(m o) -> m o", o=1))
            cb2_sb = const.tile([128, 1], F32, tag="cb2")
            for q in range(2):
                nc.sync.dma_start(out=cb2_sb[64 * q:64 * q + 64, :],
                                  in_=cb2.rearrange("(m o) -> m o", o=1))
            cb3s = const.tile([3, 1], F32, tag="cb3")
            nc.sync.dma_start(out=cb3s,
                              in_=cb3.rearrange("(m o) -> m o", o=1))
            nc.vector.tensor_scalar_mul(cb3s, cb3s, 0.1)

            # persistent activations
            zT = const.tile([32, ROWS], BF16, tag="zT")        # [32, (b j)]
            pos3 = const.tile([128, 8, J, 3], F32, tag="pos3")
            conf_b = const.tile([128, 8, J], F32, tag="conf")
            avgz = const.tile([32, 16, BC], BF16, tag="avgz")
            y1h = [const.tile([128, BC], BF16, tag=f"y1h{q}", name=f"y1h{q}")
                   for q in range(4)]

            # DRAM scratch
            xbfD = dram.tile([ROWS, FD], BF16)
            dscr = dram.tile([ROWS], F32)

            # ---------------- Phase A: depth MLP ----------------
            with tc.tile_pool(name="xt", bufs=3) as xt_pool, \
                 tc.tile_pool(name="h1p", bufs=2) as h1_pool, \
                 tc.tile_pool(name="h2p", bufs=2) as h2_pool, \
                 tc.tile_pool(name="dscp", bufs=3) as dsc_pool, \
                 tc.tile_pool(name="psA", bufs=4, space="PSUM") as psA, \
                 tc.tile_pool(name="psZ", bufs=2, space="PSUM") as psZ, \
                 tc.tile_pool(name="psD", bufs=2, space="PSUM") as psD:

                for c in range(NCH):
                    rs = slice(c * RC, (c + 1) * RC)
                    # cast fp32 -> bf16 (DRAM->DRAM, SWDGE casts in flight)
                    nc.gpsimd.dma_start(out=xbfD[rs, :], in_=feats_flat[rs, :])
                    # transposed load via xbar: XT[p, fb, r] = X[r, fb*128+p]
                    xt = xt_pool.tile([128, 4, RC], BF16, tag="xt")
                    nc.sync.dma_start_transpose(xt, xbfD[rs, :])

                    h1t = h1_pool.tile([128, 8, RC], BF16, tag="h1")
                    for m in range(8):
                        ps = psA.tile([128, RC], F32, tag="mm")
                        for k in range(4):
                            nc.tensor.matmul(ps, w1[:, k, 128 * m:128 * m + 128],
                                             xt[:, k, :],
                                             start=(k == 0), stop=(k == 3))
                        nc.scalar.activation(out=h1t[:, m, :], in_=ps,
                                             func=AF.Relu,
                                             bias=db1_sb[:, m:m + 1])
                    # z projection (shares XT)
                    psz = psZ.tile([32, RC], F32, tag="z")
                    for k in range(4):
                        nc.tensor.matmul(psz, w1z[:, k, :], xt[:, k, :],
                                         start=(k == 0), stop=(k == 3))
                    nc.vector.tensor_copy(out=zT[:, rs], in_=psz)

                    h2t = h2_pool.tile([128, 4, RC], BF16, tag="h2")
                    for n in range(4):
                        ps = psA.tile([128, RC], F32, tag="mm")
                        for kh in range(8):
                            nc.tensor.matmul(ps, w2[:, kh, 128 * n:128 * n + 128],
                                             h1t[:, kh, :],
                                             start=(kh == 0), stop=(kh == 7))
                        nc.scalar.activation(out=h2t[:, n, :], in_=ps,
                                             func=AF.Relu,
                                             bias=db2_sb[:, n:n + 1])

                    psd = psD.tile([1, RC], F32, tag="d")
                    for kh in range(4):
                        nc.tensor.matmul(psd, w3[:, kh, :], h2t[:, kh, :],
                                         start=(kh == 0), stop=(kh == 3))
                    dsc = dsc_pool.tile([1, RC], F32, tag="dsc")
                    nc.vector.tensor_scalar(out=dsc, in0=psd,
                                            scalar1=db3_sb[0:1, 0:1],
                                            scalar2=None, op0=OP.add)
                    nc.sync.dma_start(out=dscr[rs], in_=dsc)

            # ---------------- Phase B: bone refinement ----------------
            # init pos3 = [poses_2d | depths]; load confidence
            p2d_b = const.tile([128, 8, J, 2], F32, tag="p2db")
            nc.sync.dma_start(
                out=p2d_b.rearrange("p bh j c -> p bh (j c)"),
                in_=p2d.rearrange("(bh bl) j c -> bl bh (j c)", bl=128))
            dep_b = const.tile([128, 8, J], F32, tag="depb")
            nc.sync.dma_start(
                out=dep_b,
                in_=dscr.rearrange("(bh bl j) -> bl bh j", bl=128, j=J))
            nc.vector.tensor_copy(out=pos3[:, :, :, 0:2], in_=p2d_b)
            nc.vector.tensor_copy(out=pos3[:, :, :, 2], in_=dep_b)
            nc.sync.dma_start(
                out=conf_b,
                in_=conf.rearrange("(bh bl) j -> bl bh j", bl=128))
            niter = 0 if stage in ("A", "B0") else 3

            # avgz[e] = z[parent_e] + z[child_e]  (0.5 folded into w1z)
            # bone order: (0,1..6),(5,7),(6,8),(7,9),(8,10),(5,11),(6,12),
            #             (11,13),(12,14),(13,15),(14,16)
            zv = zT.rearrange("p (b j) -> p j b", j=J)     # [32, J, BC]
            groups = [(0, 6, slice(1, 7), slice(0, 1)),
                      (6, 8, slice(7, 9), slice(5, 7)),
                      (8, 10, slice(9, 11), slice(7, 9)),
                      (10, 12, slice(11, 13), slice(5, 7)),
                      (12, 14, slice(13, 15), slice(11, 13)),
                      (14, 16, slice(15, 17), slice(13, 15))]
            if stage not in ("A",):
                for (e0, e1, cs, ps_) in groups:
                    n = e1 - e0
                    in1 = zv[:, ps_, :]
                    if in1.shape[1] != n:
                        in1 = in1.to_broadcast([32, n, BC])
                    nc.vector.tensor_tensor(out=avgz[:, e0:e1, :],
                                            in0=zv[:, cs, :], in1=in1,
                                            op=OP.add)
            # scatter avgz -> y1h_q [(4 bones x 32), b] via partition-moving DMA
            if stage in ("full",):
                for e in range(16):
                    q, s_ = e // 4, e % 4
                    nc.sync.dma_start(out=y1h[q][32 * s_:32 * s_ + 32, :],
                                      in_=avgz[:, e, :])
            elif stage == "noy1h":
                for q in range(4):
                    nc.vector.memset(y1h[q], 0.0)

            with tc.tile_pool(name="pb", bufs=2) as pb, \
                 tc.tile_pool(name="g1p", bufs=2) as g1p, \
                 tc.tile_pool(name="g2p", bufs=2) as g2p, \
                 tc.tile_pool(name="psB", bufs=3, space="PSUM") as psB, \
                 tc.tile_pool(name="psPU", bufs=1, space="PSUM") as psPU, \
                 tc.tile_pool(name="psTR", bufs=2, space="PSUM") as psTR:

                for it in range(niter):
                    # ---- geometry (batch-on-partition, fp32) ----
                    bv = pb.tile([128, 8, 16, 3], F32, tag="bv")
                    for (e0, e1, cs, ps_) in groups:
                        n = e1 - e0
                        in1 = pos3[:, :, ps_, :]
                        if in1.shape[2] != n:
                            in1 = in1.to_broadcast([128, 8, n, 3])
                        nc.vector.tensor_tensor(out=bv[:, :, e0:e1, :],
                                                in0=pos3[:, :, cs, :], in1=in1,
                                                op=OP.subtract)
                    sq = pb.tile([128, 8, 16, 3], F32, tag="sq")
                    nc.vector.tensor_tensor(out=sq, in0=bv, in1=bv, op=OP.mult)
                    lensq = pb.tile([128, 8, 16], F32, tag="lensq")
                    nc.vector.tensor_reduce(out=lensq, in_=sq,
                                            axis=mybir.AxisListType.X,
                                            op=OP.add)
                    dl = pb.tile([128, 8, 16, 4], BF16, tag="dl")
                    nc.scalar.activation(out=dl[:, :, :, 3], in_=lensq,
                                         func=AF.Sqrt)
                    inv = pb.tile([128, 8, 16], F32, tag="inv")
                    nc.vector.tensor_scalar(out=inv, in0=dl[:, :, :, 3],
                                            scalar1=EPS, scalar2=None,
                                            op0=OP.add)
                    nc.vector.reciprocal(inv, inv)
                    nc.vector.tensor_tensor(
                        out=dl[:, :, :, 0:3], in0=bv,
                        in1=inv[:, :, :, None].to_broadcast([128, 8, 16, 3]),
                        op=OP.mult)

                    # transpose dirlen to [(e,4), b] layout
                    dlT = pb.tile([64, 8, 128], BF16, tag="dlT")
                    for bh in range(8):
                        pst = psTR.tile([128, 128], BF16, tag="tr")
                        nc.tensor.transpose(pst[:64, :], dl[:, bh], id_bf)
                        nc.vector.tensor_copy(out=dlT[:, bh, :],
                                              in_=pst[:64, :])

                    # ---- bone MLP (transposed layout) ----
                    g1 = [g1p.tile([128, 8, 128], BF16, tag=f"g1_{q}", name=f"g1_{q}_{it}")
                          for q in range(4)]
                    for q in range(4):
                        for hh in range(2):
                            bs = slice(hh * 4, hh * 4 + 4)
                            ps = psB.tile([128, RC], F32, tag="mm")
                            nc.tensor.matmul(
                                ps, blkW1[:, 128 * q:128 * q + 128],
                                dlT[:, bs, :],
                                start=True, stop=False)
                            nc.tensor.matmul(
                                ps, id_bf, y1h[q][:, hh * 512:hh * 512 + 512],
                                start=False, stop=True)
                            nc.scalar.activation(out=g1[q][:, bs, :], in_=ps,
                                                 func=AF.Relu, bias=cb1_sb)
                    g2 = [g2p.tile([128, 8, 128], BF16, tag=f"g2_{c2}", name=f"g2_{c2}_{it}")
                          for c2 in range(8)]
                    for c2 in range(8):
                        q, half = c2 // 2, c2 % 2
                        for hh in range(2):
                            bs = slice(hh * 4, hh * 4 + 4)
                            ps = psB.tile([128, RC], F32, tag="mm")
                            nc.tensor.matmul(
                                ps, blkW2[:, 128 * half:128 * half + 128],
                                g1[q][:, bs, :],
                                start=True, stop=True)
                            nc.scalar.activation(out=g2[c2][:, bs, :], in_=ps,
                                                 func=AF.Relu, bias=cb2_sb)
                    pu_sb = pb.tile([3, 8, 128], F32, tag="pu_sb")
                    for hh in range(2):
                        bs = slice(hh * 4, hh * 4 + 4)
                        psu = psPU.tile([3, RC], F32, tag="pu")
                        for c2 in range(8):
                            nc.tensor.matmul(psu, w3stk, g2[c2][:, bs, :],
                                             start=(c2 == 0), stop=(c2 == 7))
                        # 0.1 * (sum/16 + cb3) = sum*(0.1/16) + 0.1*cb3
                        nc.scalar.activation(out=pu_sb[:, bs, :], in_=psu,
                                             func=AF.Identity, bias=cb3s,
                                             scale=0.1 / 16.0)
                    # transpose update back to batch-on-partition layout
                    pu_b = pb.tile([128, 8, 3], F32, tag="pu_b")
                    for bh in range(8):
                        pst = psTR.tile([128, 4], F32, tag="trpu")
                        nc.tensor.transpose(pst[:, :3], pu_sb[:, bh, :],
                                            id3)
                        nc.vector.tensor_copy(out=pu_b[:, bh, :],
                                              in_=pst[:, :3])
                    # pos3 = (pos3 + pu) * conf
                    nc.vector.tensor_tensor(
                        out=pos3, in0=pos3,
                        in1=pu_b[:, :, None, :].to_broadcast([128, 8, J, 3]),
                        op=OP.add)
                    nc.vector.tensor_tensor(
                        out=pos3, in0=pos3,
                        in1=conf_b[:, :, :, None].to_broadcast([128, 8, J, 3]),
                        op=OP.mult)

            nc.sync.dma_start(
                out=out.rearrange("(bh bl) j c -> bl bh j c", bl=128),
                in_=pos3)

    nc.compile()
    return nc


import os as _os
def _get_nc():
    stage = _os.environ.get("KITRO_STAGE", "full")
    key = f"nc_{stage}"
    if key not in _CACHE:
        _CACHE[key] = _build_nc(stage)
    return _CACHE[key]


def _in_maps(inputs):
    maps = []
    for c in range(NCORE):
        bs = slice(c * BC, (c + 1) * BC)
        m = {
            "features": np.ascontiguousarray(inputs["features"][bs]),
            "poses_2d": np.ascontiguousarray(inputs["poses_2d"][bs]),
            "confidence": np.ascontiguousarray(inputs["confidence"][bs]),
        }
        for k in ("dW1", "db1", "dW2", "db2", "dW3", "db3",
                  "cW1", "cb1", "cW2", "cb2", "cW3", "cb3"):
            m[k] = np.ascontiguousarray(inputs[k])
        maps.append(m)
    return maps


def _run(inputs, **kw):
    nc = _get_nc()
    res = run_bass_kernel_spmd(nc, _in_maps(inputs),
                               core_ids=list(range(NCORE)), **kw)
    full = np.concatenate([res.results[c]["out"] for c in range(NCORE)],
                          axis=0)
    return full.astype(np.float32), res


def kernel(**inputs) -> np.ndarray:
    out, _ = _run(inputs)
    return out



# revision 2
# speedup vs baseline: 1.0002x; 1.0002x over previous
"""Trainium2 Bass kernel for nn_KITRO (gnn_message_passing).

Pure data parallel over 8 NeuronCores: batch 8192 -> 1024 per core.
Per core:
  Phase A (depth MLP, transposed dataflow, fp8e4 DoubleRow matmuls /
  fp32 accum; weights pre-scaled x16 so fp8 stays in normal range,
  scales folded into biases and PSUM evictions):
    features f32 --cast DMA--> bf16 DRAM scratch --xbar transpose DMA-->
    XT bf16 --DVE cast--> XT fp8 [512f, rows] in SBUF;
    h1T = relu(W1^T XT + b1) (stored 16*h1, fp8);
    h2T = relu(W2^T h1T + b2) (stored 16*h2, fp8);
    depths = W3^T h2T + b3 -> DRAM scratch.
    Also zT = (0.5*cW1[4:])^T XT (bone-feature projection, averaged later
    in 32-dim z space instead of 512-dim feature space -- exact).
  Phase B (3 bone refinement iterations):
    batch-on-partition geometry (bone vectors / length / direction) on
    DVE/ACT, tiny bone MLP via block-diagonal matmuls in transposed
    layout on PE, per-batch mean over bones fused into the last matmul.
"""

import sys

if "/opt/trn_rl_repo" not in sys.path:
    sys.path.insert(0, "/opt/trn_rl_repo")

import contextlib

import numpy as np

import concourse.bass as bass
import concourse.mybir as mybir
import concourse.tile as tile
from concourse import bacc
from concourse.bass_utils import run_bass_kernel_spmd
from concourse.masks import make_identity

F32 = mybir.dt.float32
BF16 = mybir.dt.bfloat16
FP8 = mybir.dt.float8e4
AF = mybir.ActivationFunctionType
OP = mybir.AluOpType
DR = mybir.MatmulPerfMode.DoubleRow

NCORE = 8
B = 8192
BC = B // NCORE          # 1024 batches per core
J = 25
FD = 512
HD = 1024
ROWS = BC * J            # 25600 rows per core
RC = 512                 # row-chunk
NCH = ROWS // RC         # 50 chunks
EPS = 1e-8
WS = 16.0                # fp8 weight pre-scale

_CACHE = {}


def _build_nc(stage="full", rep=1):
    nc = bacc.Bacc("TRN2", target_bir_lowering=False, debug=False,
                   num_devices=NCORE)

    # ---- per-core DRAM I/O ----
    feats = nc.dram_tensor("features", [BC, J, FD], F32, kind="ExternalInput")
    p2d = nc.dram_tensor("poses_2d", [BC, J, 2], F32, kind="ExternalInput")
    conf = nc.dram_tensor("confidence", [BC, J], F32, kind="ExternalInput")
    dW1 = nc.dram_tensor("dW1", [FD, HD], F32, kind="ExternalInput")
    db1 = nc.dram_tensor("db1", [HD], F32, kind="ExternalInput")
    dW2 = nc.dram_tensor("dW2", [HD, FD], F32, kind="ExternalInput")
    db2 = nc.dram_tensor("db2", [FD], F32, kind="ExternalInput")
    dW3 = nc.dram_tensor("dW3", [FD, 1], F32, kind="ExternalInput")
    db3 = nc.dram_tensor("db3", [1], F32, kind="ExternalInput")
    cW1 = nc.dram_tensor("cW1", [4 + FD, 32], F32, kind="ExternalInput")
    cb1 = nc.dram_tensor("cb1", [32], F32, kind="ExternalInput")
    cW2 = nc.dram_tensor("cW2", [32, 64], F32, kind="ExternalInput")
    cb2 = nc.dram_tensor("cb2", [64], F32, kind="ExternalInput")
    cW3 = nc.dram_tensor("cW3", [64, 3], F32, kind="ExternalInput")
    cb3 = nc.dram_tensor("cb3", [3], F32, kind="ExternalInput")
    out = nc.dram_tensor("out", [BC, J, 3], F32, kind="ExternalOutput")

    (feats, p2d, conf, dW1, db1, dW2, db2, dW3, db3,
     cW1, cb1, cW2, cb2, cW3, cb3, out) = (
        t.ap() for t in (feats, p2d, conf, dW1, db1, dW2, db2, dW3, db3,
                         cW1, cb1, cW2, cb2, cW3, cb3, out))

    feats_flat = feats.flatten_outer_dims()          # [ROWS, FD]

    with tile.TileContext(nc) as tc:
        with contextlib.ExitStack() as ctx:
            const = ctx.enter_context(tc.tile_pool(name="const", bufs=1))
            dram = ctx.enter_context(
                tc.tile_pool(name="dram", bufs=1, space="DRAM"))

            # ---- constants / weights ----
            id_bf = const.tile([128, 128], BF16, tag="id")
            make_identity(nc, id_bf)
            id3 = const.tile([3, 3], F32, tag="id3")
            make_identity(nc, id3)

            # fp8 weights, pre-scaled by WS (=16) via f32 staging + DVE cast
            w1 = const.tile([128, 4, HD], FP8, tag="w1")
            w1z = const.tile([128, 4, 32], FP8, tag="w1z")
            w2 = const.tile([128, 8, FD], FP8, tag="w2")
            w3p = const.tile([128, 4, 16], FP8, tag="w3p")
            with tc.tile_pool(name="wstage", bufs=1) as wst:
                st1 = wst.tile([128, 4, HD], F32, tag="st1")
                nc.sync.dma_start(
                    out=st1, in_=dW1.rearrange("(kh p) m -> p kh m", p=128))
                nc.vector.tensor_scalar_mul(w1, st1, WS)
                st2 = wst.tile([128, 8, FD], F32, tag="st2")
                nc.sync.dma_start(
                    out=st2, in_=dW2.rearrange("(kh p) n -> p kh n", p=128))
                nc.vector.tensor_scalar_mul(w2, st2, WS)
                stz = wst.tile([128, 4, 32], F32, tag="stz")
                nc.sync.dma_start(
                    out=stz, in_=cW1[4:].rearrange("(kh p) m -> p kh m", p=128))
                # fold the 0.5 bone-average factor into the z projection
                nc.vector.tensor_scalar_mul(w1z, stz, 0.5 * WS)
                st3 = wst.tile([128, 4, 1], F32, tag="st3")
                nc.sync.dma_start(
                    out=st3, in_=dW3.rearrange("(kh p) o -> p kh o", p=128))
                nc.vector.memset(w3p, 0.0)
                nc.vector.tensor_scalar_mul(w3p[:, :, 0:1], st3, WS)

            # block-diagonal bone weights (bf16, Phase B)
            blkW1 = const.tile([64, 512], BF16, tag="blkW1")   # 16x cW1[:4]
            nc.vector.memset(blkW1, 0.0)
            for d in range(16):
                nc.gpsimd.dma_start(
                    out=blkW1[4 * d:4 * d + 4, 32 * d:32 * d + 32],
                    in_=cW1[0:4, :])
            blkW2 = const.tile([128, 256], BF16, tag="blkW2")  # 4x cW2
            nc.vector.memset(blkW2, 0.0)
            for d in range(4):
                nc.gpsimd.dma_start(
                    out=blkW2[32 * d:32 * d + 32, 64 * d:64 * d + 64],
                    in_=cW2[:, :])
            w3stk = const.tile([128, 3], BF16, tag="w3stk")    # cW3 stacked 2x
            for d in range(2):
                nc.gpsimd.dma_start(out=w3stk[64 * d:64 * d + 64, :],
                                    in_=cW3[:, :])

            # biases (depth-MLP biases pre-scaled by WS to match weights)
            db1_sb = const.tile([128, 8], F32, tag="db1")
            nc.sync.dma_start(out=db1_sb,
                              in_=db1.rearrange("(m p) -> p m", p=128))
            nc.vector.tensor_scalar_mul(db1_sb, db1_sb, WS)
            db2_sb = const.tile([128, 4], F32, tag="db2")
            nc.sync.dma_start(out=db2_sb,
                              in_=db2.rearrange("(m p) -> p m", p=128))
            nc.vector.tensor_scalar_mul(db2_sb, db2_sb, WS)
            db3_sb = const.tile([1, 1], F32, tag="db3")
            nc.sync.dma_start(out=db3_sb,
                              in_=db3.rearrange("(a o) -> a o", a=1))
            cb1_sb = const.tile([128, 1], F32, tag="cb1")
            for q in range(4):
                nc.sync.dma_start(out=cb1_sb[32 * q:32 * q + 32, :],
                                  in_=cb1.rearrange("(m o) -> m o", o=1))
            cb2_sb = const.tile([128, 1], F32, tag="cb2")
            for q in range(2):
                nc.sync.dma_start(out=cb2_sb[64 * q:64 * q + 64, :],
                                  in_=cb2.rearrange("(m o) -> m o", o=1))
            cb3s = const.tile([3, 1], F32, tag="cb3")
            nc.sync.dma_start(out=cb3s,
                              in_=cb3.rearrange("(m o) -> m o", o=1))
            nc.vector.tensor_scalar_mul(cb3s, cb3s, 0.1)

            # persistent activations
            zT = const.tile([32, ROWS], BF16, tag="zT")        # [32, (b j)]
            pos3 = const.tile([128, 8, J, 3], F32, tag="pos3")
            conf_b = const.tile([128, 8, J], F32, tag="conf")
            avgz = const.tile([32, 16, BC], BF16, tag="avgz")
            y1h = [const.tile([128, BC], BF16, tag=f"y1h{q}", name=f"y1h{q}")
                   for q in range(4)]

            # DRAM scratch
            xbfD = dram.tile([ROWS, FD], BF16)
            dscr = dram.tile([ROWS], F32)

            rep_ctx = tc.For_i(0, rep) if rep > 1 else contextlib.nullcontext()
            with rep_ctx:
                # ---------------- Phase A: depth MLP ----------------
                with tc.tile_pool(name="xt", bufs=3) as xt_pool, \
                     tc.tile_pool(name="xt8p", bufs=3) as xt8_pool, \
                     tc.tile_pool(name="h1p", bufs=2) as h1_pool, \
                     tc.tile_pool(name="h2p", bufs=2) as h2_pool, \
                     tc.tile_pool(name="dscp", bufs=3) as dsc_pool, \
                     tc.tile_pool(name="psA", bufs=4, space="PSUM") as psA, \
                     tc.tile_pool(name="psZ", bufs=2, space="PSUM") as psZ, \
                     tc.tile_pool(name="psD", bufs=2, space="PSUM") as psD:

                    for c in range(NCH):
                        rs = slice(c * RC, (c + 1) * RC)
                        # cast fp32 -> bf16 (DRAM->DRAM, SWDGE casts in flight)
                        nc.gpsimd.dma_start(out=xbfD[rs, :],
                                            in_=feats_flat[rs, :])
                        # transposed load via xbar: XT[p, fb, r]
                        xt = xt_pool.tile([128, 4, RC], BF16, tag="xt")
                        nc.sync.dma_start_transpose(xt, xbfD[rs, :])
                        xt8 = xt8_pool.tile([128, 4, RC], FP8, tag="xt8")
                        nc.vector.tensor_copy(out=xt8, in_=xt)

                        h1t = h1_pool.tile([128, 8, RC], FP8, tag="h1")
                        for m in range(8):
                            ps = psA.tile([128, RC], F32, tag="mm")
                            for kp in range(2):
                                nc.tensor.matmul(
                                    ps, w1[:, 2 * kp:2 * kp + 2,
                                           128 * m:128 * m + 128],
                                    xt8[:, 2 * kp:2 * kp + 2, :],
                                    start=(kp == 0), stop=(kp == 1),
                                    perf_mode=DR)
                            # psum = 16*(x@W1); out = relu(ps + 16*b1) = 16*h1
                            nc.scalar.activation(out=h1t[:, m, :], in_=ps,
                                                 func=AF.Relu,
                                                 bias=db1_sb[:, m:m + 1])
                        # z projection (shares XT); psum = 16*z
                        psz = psZ.tile([32, RC], F32, tag="z")
                        for kp in range(2):
                            nc.tensor.matmul(
                                psz, w1z[:, 2 * kp:2 * kp + 2, :],
                                xt8[:, 2 * kp:2 * kp + 2, :],
                                start=(kp == 0), stop=(kp == 1), perf_mode=DR)
                        nc.vector.tensor_scalar(out=zT[:, rs], in0=psz,
                                                scalar1=1.0 / WS, scalar2=None,
                                                op0=OP.mult)

                        h2t = h2_pool.tile([128, 4, RC], FP8, tag="h2")
                        for n in range(4):
                            ps = psA.tile([128, RC], F32, tag="mm")
                            for kp in range(4):
                                nc.tensor.matmul(
                                    ps, w2[:, 2 * kp:2 * kp + 2,
                                           128 * n:128 * n + 128],
                                    h1t[:, 2 * kp:2 * kp + 2, :],
                                    start=(kp == 0), stop=(kp == 3),
                                    perf_mode=DR)
                            # psum = 256*(h1@W2); relu(ps/16 + 16*b2) = 16*h2
                            nc.scalar.activation(out=h2t[:, n, :], in_=ps,
                                                 func=AF.Relu, scale=1.0 / WS,
                                                 bias=db2_sb[:, n:n + 1])

                        psd = psD.tile([1, RC], F32, tag="d")
                        for kp in range(2):
                            nc.tensor.matmul(
                                psd, w3p[:, 2 * kp:2 * kp + 2, 0:1],
                                h2t[:, 2 * kp:2 * kp + 2, :],
                                start=(kp == 0), stop=(kp == 1), perf_mode=DR)
                        dsc = dsc_pool.tile([1, RC], F32, tag="dsc")
                        # psum = 256*d_raw; d = ps/256 + b3
                        nc.vector.tensor_scalar(out=dsc, in0=psd,
                                                scalar1=1.0 / (WS * WS),
                                                scalar2=db3_sb[0:1, 0:1],
                                                op0=OP.mult, op1=OP.add)
                        nc.sync.dma_start(out=dscr[rs], in_=dsc)

                # ---------------- Phase B: bone refinement ----------------
                # init pos3 = [poses_2d | depths]; load confidence
                p2d_b = const.tile([128, 8, J, 2], F32, tag="p2db")
                nc.sync.dma_start(
                    out=p2d_b.rearrange("p bh j c -> p bh (j c)"),
                    in_=p2d.rearrange("(bh bl) j c -> bl bh (j c)", bl=128))
                dep_b = const.tile([128, 8, J], F32, tag="depb")
                nc.sync.dma_start(
                    out=dep_b,
                    in_=dscr.rearrange("(bh bl j) -> bl bh j", bl=128, j=J))
                nc.vector.tensor_copy(out=pos3[:, :, :, 0:2], in_=p2d_b)
                nc.vector.tensor_copy(out=pos3[:, :, :, 2], in_=dep_b)
                nc.sync.dma_start(
                    out=conf_b,
                    in_=conf.rearrange("(bh bl) j -> bl bh j", bl=128))
                niter = 0 if stage in ("A", "B0") else 3

                # avgz[e] = z[parent_e] + z[child_e]  (0.5 folded into w1z)
                zv = zT.rearrange("p (b j) -> p j b", j=J)     # [32, J, BC]
                groups = [(0, 6, slice(1, 7), slice(0, 1)),
                          (6, 8, slice(7, 9), slice(5, 7)),
                          (8, 10, slice(9, 11), slice(7, 9)),
                          (10, 12, slice(11, 13), slice(5, 7)),
                          (12, 14, slice(13, 15), slice(11, 13)),
                          (14, 16, slice(15, 17), slice(13, 15))]
                if stage not in ("A",):
                    for (e0, e1, cs, ps_) in groups:
                        n = e1 - e0
                        in1 = zv[:, ps_, :]
                        if in1.shape[1] != n:
                            in1 = in1.to_broadcast([32, n, BC])
                        nc.vector.tensor_tensor(out=avgz[:, e0:e1, :],
                                                in0=zv[:, cs, :], in1=in1,
                                                op=OP.add)
                # scatter avgz -> y1h_q [(4 bones x 32), b]
                if stage in ("full",):
                    for e in range(16):
                        q, s_ = e // 4, e % 4
                        nc.sync.dma_start(out=y1h[q][32 * s_:32 * s_ + 32, :],
                                          in_=avgz[:, e, :])
                elif stage == "noy1h":
                    for q in range(4):
                        nc.vector.memset(y1h[q], 0.0)

                with tc.tile_pool(name="pb", bufs=2) as pb, \
                     tc.tile_pool(name="g1p", bufs=2) as g1p, \
                     tc.tile_pool(name="g2p", bufs=2) as g2p, \
                     tc.tile_pool(name="psB", bufs=3, space="PSUM") as psB, \
                     tc.tile_pool(name="psPU", bufs=1, space="PSUM") as psPU, \
                     tc.tile_pool(name="psTR", bufs=2, space="PSUM") as psTR:

                    for it in range(niter):
                        # ---- geometry (batch-on-partition, fp32) ----
                        bv = pb.tile([128, 8, 16, 3], F32, tag="bv")
                        for (e0, e1, cs, ps_) in groups:
                            n = e1 - e0
                            in1 = pos3[:, :, ps_, :]
                            if in1.shape[2] != n:
                                in1 = in1.to_broadcast([128, 8, n, 3])
                            nc.vector.tensor_tensor(out=bv[:, :, e0:e1, :],
                                                    in0=pos3[:, :, cs, :],
                                                    in1=in1, op=OP.subtract)
                        sq = pb.tile([128, 8, 16, 3], F32, tag="sq")
                        nc.vector.tensor_tensor(out=sq, in0=bv, in1=bv,
                                                op=OP.mult)
                        lensq = pb.tile([128, 8, 16], F32, tag="lensq")
                        nc.vector.tensor_reduce(out=lensq, in_=sq,
                                                axis=mybir.AxisListType.X,
                                                op=OP.add)
                        dl = pb.tile([128, 8, 16, 4], BF16, tag="dl")
                        nc.scalar.activation(out=dl[:, :, :, 3], in_=lensq,
                                             func=AF.Sqrt)
                        inv = pb.tile([128, 8, 16], F32, tag="inv")
                        nc.vector.tensor_scalar(out=inv, in0=dl[:, :, :, 3],
                                                scalar1=EPS, scalar2=None,
                                                op0=OP.add)
                        nc.vector.reciprocal(inv, inv)
                        nc.vector.tensor_tensor(
                            out=dl[:, :, :, 0:3], in0=bv,
                            in1=inv[:, :, :, None].to_broadcast([128, 8, 16, 3]),
                            op=OP.mult)

                        # transpose dirlen to [(e,4), b] layout
                        dlT = pb.tile([64, 8, 128], BF16, tag="dlT")
                        for bh in range(8):
                            pst = psTR.tile([128, 128], BF16, tag="tr")
                            nc.tensor.transpose(pst[:64, :], dl[:, bh], id_bf)
                            nc.vector.tensor_copy(out=dlT[:, bh, :],
                                                  in_=pst[:64, :])

                        # ---- bone MLP (transposed layout) ----
                        g1 = [g1p.tile([128, 8, 128], BF16, tag=f"g1_{q}",
                                       name=f"g1_{q}_{it}")
                              for q in range(4)]
                        for q in range(4):
                            for hh in range(2):
                                bs = slice(hh * 4, hh * 4 + 4)
                                ps = psB.tile([128, RC], F32, tag="mm")
                                nc.tensor.matmul(
                                    ps, blkW1[:, 128 * q:128 * q + 128],
                                    dlT[:, bs, :],
                                    start=True, stop=False)
                                nc.tensor.matmul(
                                    ps, id_bf,
                                    y1h[q][:, hh * 512:hh * 512 + 512],
                                    start=False, stop=True)
                                nc.scalar.activation(out=g1[q][:, bs, :],
                                                     in_=ps, func=AF.Relu,
                                                     bias=cb1_sb)
                        g2 = [g2p.tile([128, 8, 128], BF16, tag=f"g2_{c2}",
                                       name=f"g2_{c2}_{it}")
                              for c2 in range(8)]
                        for c2 in range(8):
                            q, half = c2 // 2, c2 % 2
                            for hh in range(2):
                                bs = slice(hh * 4, hh * 4 + 4)
                                ps = psB.tile([128, RC], F32, tag="mm")
                                nc.tensor.matmul(
                                    ps, blkW2[:, 128 * half:128 * half + 128],
                                    g1[q][:, bs, :],
                                    start=True, stop=True)
                                nc.scalar.activation(out=g2[c2][:, bs, :],
                                                     in_=ps, func=AF.Relu,
                                                     bias=cb2_sb)
                        pu_sb = pb.tile([3, 8, 128], F32, tag="pu_sb")
                        for hh in range(2):
                            bs = slice(hh * 4, hh * 4 + 4)
                            psu = psPU.tile([3, RC], F32, tag="pu")
                            for c2 in range(8):
                                nc.tensor.matmul(psu, w3stk, g2[c2][:, bs, :],
                                                 start=(c2 == 0),
                                                 stop=(c2 == 7))
                            # 0.1 * (sum/16 + cb3) = sum*(0.1/16) + 0.1*cb3
                            nc.scalar.activation(out=pu_sb[:, bs, :], in_=psu,
                                                 func=AF.Identity, bias=cb3s,
                                                 scale=0.1 / 16.0)
                        # transpose update back to batch-on-partition layout
                        pu_b = pb.tile([128, 8, 3], F32, tag="pu_b")
                        for bh in range(8):
                            pst = psTR.tile([128, 4], F32, tag="trpu")
                            nc.tensor.transpose(pst[:, :3], pu_sb[:, bh, :],
                                                id3)
                            nc.vector.tensor_copy(out=pu_b[:, bh, :],
                                                  in_=pst[:, :3])
                        # pos3 = (pos3 + pu) * conf
                        nc.vector.tensor_tensor(
                            out=pos3, in0=pos3,
                            in1=pu_b[:, :, None, :].to_broadcast([128, 8, J, 3]),
                            op=OP.add)
                        nc.vector.tensor_tensor(
                            out=pos3, in0=pos3,
                            in1=conf_b[:, :, :, None].to_broadcast(
                                [128, 8, J, 3]),
                            op=OP.mult)

                nc.sync.dma_start(
                    out=out.rearrange("(bh bl) j c -> bl bh j c", bl=128),
                    in_=pos3)

    nc.compile()
    return nc


import os as _os
def _get_nc():
    stage = _os.environ.get("KITRO_STAGE", "full")
    rep = int(_os.environ.get("KITRO_REPEAT", "1"))
    key = f"nc_{stage}_{rep}"
    if key not in _CACHE:
        _CACHE[key] = _build_nc(stage, rep)
    return _CACHE[key]


def _in_maps(inputs):
    maps = []
    for c in range(NCORE):
        bs = slice(c * BC, (c + 1) * BC)
        m = {
            "features": np.ascontiguousarray(inputs["features"][bs]),
            "poses_2d": np.ascontiguousarray(inputs["poses_2d"][bs]),
            "confidence": np.ascontiguousarray(inputs["confidence"][bs]),
        }
        for k in ("dW1", "db1", "dW2", "db2", "dW3", "db3",
                  "cW1", "cb1", "cW2", "cb2", "cW3", "cb3"):
            m[k] = np.ascontiguousarray(inputs[k])
        maps.append(m)
    return maps


def _run(inputs, **kw):
    nc = _get_nc()
    res = run_bass_kernel_spmd(nc, _in_maps(inputs),
                               core_ids=list(range(NCORE)), **kw)
    full = np.concatenate([res.results[c]["out"] for c in range(NCORE)],
                          axis=0)
    return full.astype(np.float32), res


def kernel(**inputs) -> np.ndarray:
    out, _ = _run(inputs)
    return out


# revision 23
# speedup vs baseline: 1.1954x; 1.1952x over previous
"""Trainium2 Bass kernel for nn_KITRO (gnn_message_passing).

Pure data parallel over 8 NeuronCores: batch 8192 -> 1024 per core.
Per core:
  Phase A (depth MLP, transposed dataflow, fp8e4 DoubleRow matmuls /
  fp32 accum; weights pre-scaled x16 so fp8 stays in normal range,
  scales folded into biases and PSUM evictions):
    features f32 --cast DMA--> bf16 DRAM scratch --xbar transpose DMA-->
    XT bf16 --DVE cast--> XT fp8 [512f, rows] in SBUF;
    h1T = relu(W1^T XT + b1) (stored 16*h1, fp8);
    h2T = relu(W2^T h1T + b2) (stored 16*h2, fp8);
    depths = W3^T h2T + b3 -> DRAM scratch.
    Also zT = (0.5*cW1[4:])^T XT (bone-feature projection, averaged later
    in 32-dim z space instead of 512-dim feature space -- exact).
  Phase B (3 bone refinement iterations):
    batch-on-partition geometry (bone vectors / length / direction) on
    DVE/ACT, tiny bone MLP via block-diagonal matmuls in transposed
    layout on PE, per-batch mean over bones fused into the last matmul.
"""

import sys

if "/opt/trn_rl_repo" not in sys.path:
    sys.path.insert(0, "/opt/trn_rl_repo")

import contextlib

import os as _os

import numpy as np

import concourse.bass as bass
import concourse.mybir as mybir
import concourse.tile as tile
from concourse import bacc
from concourse.bass_utils import run_bass_kernel_spmd
from concourse.masks import make_identity

F32 = mybir.dt.float32
BF16 = mybir.dt.bfloat16
FP8 = mybir.dt.float8e4
AF = mybir.ActivationFunctionType
OP = mybir.AluOpType
DR = mybir.MatmulPerfMode.DoubleRow

NCORE = 8
B = 8192
BC = B // NCORE          # 1024 batches per core
J = 25
FD = 512
HD = 1024
ROWS = BC * J            # 25600 rows per core
RC = 512                 # row-chunk
NCH = ROWS // RC         # 50 chunks
EPS = 1e-8
WS = 8.0                 # fp8 weight pre-scale

_CACHE = {}


def _build_nc(stage="full", rep=1):
    nc = bacc.Bacc("TRN2", target_bir_lowering=False, debug=False,
                   num_devices=NCORE)

    # ---- per-core DRAM I/O ----
    feats = nc.dram_tensor("features", [BC, J, FD], F32, kind="ExternalInput")
    p2d = nc.dram_tensor("poses_2d", [BC, J, 2], F32, kind="ExternalInput")
    conf = nc.dram_tensor("confidence", [BC, J], F32, kind="ExternalInput")
    dW1 = nc.dram_tensor("dW1", [FD, HD], F32, kind="ExternalInput")
    db1 = nc.dram_tensor("db1", [HD], F32, kind="ExternalInput")
    dW2 = nc.dram_tensor("dW2", [HD, FD], F32, kind="ExternalInput")
    db2 = nc.dram_tensor("db2", [FD], F32, kind="ExternalInput")
    dW3 = nc.dram_tensor("dW3", [FD, 1], F32, kind="ExternalInput")
    db3 = nc.dram_tensor("db3", [1], F32, kind="ExternalInput")
    cW1 = nc.dram_tensor("cW1", [4 + FD, 32], F32, kind="ExternalInput")
    cb1 = nc.dram_tensor("cb1", [32], F32, kind="ExternalInput")
    cW2 = nc.dram_tensor("cW2", [32, 64], F32, kind="ExternalInput")
    cb2 = nc.dram_tensor("cb2", [64], F32, kind="ExternalInput")
    cW3 = nc.dram_tensor("cW3", [64, 3], F32, kind="ExternalInput")
    cb3 = nc.dram_tensor("cb3", [3], F32, kind="ExternalInput")
    out = nc.dram_tensor("out", [BC, J, 3], F32, kind="ExternalOutput")

    (feats, p2d, conf, dW1, db1, dW2, db2, dW3, db3,
     cW1, cb1, cW2, cb2, cW3, cb3, out) = (
        t.ap() for t in (feats, p2d, conf, dW1, db1, dW2, db2, dW3, db3,
                         cW1, cb1, cW2, cb2, cW3, cb3, out))

    feats_flat = feats.flatten_outer_dims()          # [ROWS, FD]

    with tile.TileContext(nc) as tc:
        with contextlib.ExitStack() as ctx:
            const = ctx.enter_context(tc.tile_pool(name="const", bufs=1))
            dram = ctx.enter_context(
                tc.tile_pool(name="dram", bufs=1, space="DRAM"))

            # ---- constants / weights ----
            id_bf = const.tile([128, 128], BF16, tag="id")
            make_identity(nc, id_bf)
            id3_bf = const.tile([3, 3], BF16, tag="id3")
            make_identity(nc, id3_bf)

            # fp8 weights, pre-scaled by WS (=16) via f32 staging + DVE cast
            w1 = const.tile([128, 4, HD], FP8, tag="w1")
            w1z = const.tile([128, 4, 32], FP8, tag="w1z")
            w2 = const.tile([128, 8, FD], FP8, tag="w2")
            w3p = const.tile([128, 4, 16], FP8, tag="w3p")
            # weight staging DMAs ride the Activation HWDGE queue so they
            # don't serialize with the per-chunk transpose loads on SP
            with tc.tile_pool(name="wstage", bufs=1) as wst:
                st1 = wst.tile([128, 4, HD], F32, tag="st1")
                nc.scalar.dma_start(
                    out=st1, in_=dW1.rearrange("(kh p) m -> p kh m", p=128))
                nc.vector.tensor_scalar_mul(w1, st1, WS)
                st2 = wst.tile([128, 8, FD], F32, tag="st2")
                nc.scalar.dma_start(
                    out=st2, in_=dW2.rearrange("(kh p) n -> p kh n", p=128))
                nc.vector.tensor_scalar_mul(w2, st2, WS)
                stz = wst.tile([128, 4, 32], F32, tag="stz")
                nc.scalar.dma_start(
                    out=stz, in_=cW1[4:].rearrange("(kh p) m -> p kh m", p=128))
                # fold the 0.5 bone-average factor into the z projection
                nc.vector.tensor_scalar_mul(w1z, stz, 0.5 * WS)
                st3 = wst.tile([128, 4, 1], F32, tag="st3")
                nc.scalar.dma_start(
                    out=st3, in_=dW3.rearrange("(kh p) o -> p kh o", p=128))
                nc.vector.memset(w3p, 0.0)
                nc.vector.tensor_scalar_mul(w3p[:, :, 0:1], st3, WS)

            # block-diagonal bone weights (bf16, Phase B)
            # block-diagonal bone weight tiles (filled after the Phase A
            # chunk loop is emitted, so their casting SWDGE DMAs queue
            # behind the chunk-0 feature casts on the gpsimd queue)
            blkW1 = const.tile([64, 512], BF16, tag="blkW1")   # 16x cW1[:4]
            blkW2 = const.tile([128, 256], BF16, tag="blkW2")  # 4x cW2
            w3stk = const.tile([128, 3], BF16, tag="w3stk")    # cW3 stacked 2x

            def _fill_bone_weights():
                nc.vector.memset(blkW1, 0.0)
                for d in range(16):
                    nc.gpsimd.dma_start(
                        out=blkW1[4 * d:4 * d + 4, 32 * d:32 * d + 32],
                        in_=cW1[0:4, :])
                nc.vector.memset(blkW2, 0.0)
                for d in range(4):
                    nc.gpsimd.dma_start(
                        out=blkW2[32 * d:32 * d + 32, 64 * d:64 * d + 64],
                        in_=cW2[:, :])
                for d in range(2):
                    nc.gpsimd.dma_start(out=w3stk[64 * d:64 * d + 64, :],
                                        in_=cW3[:, :])

            # biases (depth-MLP biases pre-scaled by WS to match weights)
            db1_sb = const.tile([128, 8], F32, tag="db1")
            nc.sync.dma_start(out=db1_sb,
                              in_=db1.rearrange("(m p) -> p m", p=128))
            nc.vector.tensor_scalar_mul(db1_sb, db1_sb, WS)
            db2_sb = const.tile([128, 4], F32, tag="db2")
            nc.sync.dma_start(out=db2_sb,
                              in_=db2.rearrange("(m p) -> p m", p=128))
            nc.vector.tensor_scalar_mul(db2_sb, db2_sb, WS * WS)
            db3_sb = const.tile([1, 1], F32, tag="db3")
            nc.sync.dma_start(out=db3_sb,
                              in_=db3.rearrange("(a o) -> a o", a=1))
            cb1_sb = const.tile([128, 1], F32, tag="cb1")
            for q in range(4):
                nc.sync.dma_start(out=cb1_sb[32 * q:32 * q + 32, :],
                                  in_=cb1.rearrange("(m o) -> m o", o=1))
            cb2_sb = const.tile([128, 1], F32, tag="cb2")
            for q in range(2):
                nc.sync.dma_start(out=cb2_sb[64 * q:64 * q + 64, :],
                                  in_=cb2.rearrange("(m o) -> m o", o=1))
            cb3s = const.tile([3, 1], F32, tag="cb3")
            nc.sync.dma_start(out=cb3s,
                              in_=cb3.rearrange("(m o) -> m o", o=1))
            nc.vector.tensor_scalar_mul(cb3s, cb3s, 0.1)

            # persistent activations
            zT = const.tile([32, ROWS], BF16, tag="zT")        # [32, (b j)]
            pos3 = const.tile([128, 8, J, 3], F32, tag="pos3")
            conf_b = const.tile([128, 8, J], F32, tag="conf")
            avgz = const.tile([32, 16, BC], BF16, tag="avgz")
            y1h = [const.tile([128, BC], BF16, tag=f"y1h{q}", name=f"y1h{q}")
                   for q in range(4)]
            # full-tile writes so the tile allocator always sees these as
            # written before their partial-partition DMA scatters
            for q in range(4):
                nc.vector.memset(y1h[q], 0.0)

            # DRAM scratch (4 explicitly reused buffers: cast DMA c+4 must
            # wait for transpose c, keeping the SWDGE queue from running
            # 50 chunks ahead and stalling the first transposes)
            xbfs = [dram.tile([RC, FD], BF16, name=f"xbf{i}")
                    for i in range(8)]
            dscr = dram.tile([ROWS], F32)

            # skeleton bone groups: (e0, e1, child slice, parent slice)
            groups = [(0, 6, slice(1, 7), slice(0, 1)),
                      (6, 8, slice(7, 9), slice(5, 7)),
                      (8, 10, slice(9, 11), slice(7, 9)),
                      (10, 12, slice(11, 13), slice(5, 7)),
                      (12, 14, slice(13, 15), slice(11, 13)),
                      (14, 16, slice(15, 17), slice(13, 15))]

            split = _os.environ.get("KITRO_SPLIT", "2way")
            H1_EV = ["a", "v", "a", "v", "a", "v", "a", "v"]
            xt8_pool_cast = split == "3way"
            NG = int(_os.environ.get("KITRO_NG", "2"))  # batch groups (Phase B of group g
            CPG = NCH // NG          # overlaps Phase A of group g+1)
            BHG = 8 // NG            # bh blocks per group
            BG = BC // NG            # batches per group
            niter = 0 if stage in ("A", "B0") else 3

            rep_ctx = tc.For_i(0, rep) if rep > 1 else contextlib.nullcontext()
            with rep_ctx:
                with tc.tile_pool(name="xt", bufs=4) as xt_pool, \
                     tc.tile_pool(name="xt8p", bufs=4) as xt8_pool, \
                     tc.tile_pool(name="h1p", bufs=3) as h1_pool, \
                     tc.tile_pool(name="h2p", bufs=3) as h2_pool, \
                     tc.tile_pool(name="dscp", bufs=3) as dsc_pool, \
                     tc.tile_pool(name="pb", bufs=2) as pb, \
                     tc.tile_pool(name="g1p", bufs=2) as g1p, \
                     tc.tile_pool(name="g2p", bufs=2) as g2p, \
                     tc.tile_pool(name="psA", bufs=4, space="PSUM") as psA, \
                     tc.tile_pool(name="psB", bufs=1, space="PSUM") as psB, \
                     tc.tile_pool(name="psZD", bufs=1, space="PSUM") as psZD, \
                     tc.tile_pool(name="psPU", bufs=1, space="PSUM") as psPU, \
                     tc.tile_pool(name="psTR", bufs=1, space="PSUM") as psTR:

                    def phase_a_chunk(c):
                        rs = slice(c * RC, (c + 1) * RC)
                        # cast fp32 -> bf16 (DRAM->DRAM, SWDGE casts in
                        # flight; 8 rotating buffers bound queue run-ahead)
                        xbf = xbfs[c % 8]
                        nc.gpsimd.dma_start(out=xbf, in_=feats_flat[rs, :])
                        # transposed load via xbar: XT[p, fb, r]
                        xt = xt_pool.tile([128, 4, RC], BF16, tag="xt")
                        nc.sync.dma_start_transpose(xt, xbf)
                        xt8 = xt8_pool.tile([128, 4, RC], FP8, tag="xt8")
                        (nc.gpsimd if xt8_pool_cast else nc.vector).tensor_copy(
                            out=xt8, in_=xt)

                        h1t = h1_pool.tile([128, 8, RC], FP8, tag="h1")
                        for m in range(8):
                            ps = psA.tile([128, RC], F32, tag="mm")
                            for kp in range(2):
                                nc.tensor.matmul(
                                    ps, w1[:, 2 * kp:2 * kp + 2,
                                           128 * m:128 * m + 128],
                                    xt8[:, 2 * kp:2 * kp + 2, :],
                                    start=(kp == 0), stop=(kp == 1),
                                    perf_mode=DR)
                            # psum = 16*(x@W1); out = relu(ps + 16*b1) = 16*h1
                            # split evictions across ACT / DVE / Pool
                            eng = H1_EV[m]
                            if eng == "a":
                                nc.scalar.activation(out=h1t[:, m, :], in_=ps,
                                                     func=AF.Relu,
                                                     bias=db1_sb[:, m:m + 1])
                            else:
                                e = nc.vector if eng == "v" else nc.gpsimd
                                e.tensor_scalar(
                                    out=h1t[:, m, :], in0=ps,
                                    scalar1=db1_sb[:, m:m + 1], scalar2=0.0,
                                    op0=OP.add, op1=OP.max)
                        # z projection (shares XT); psum = 16*z
                        psz = psZD.tile([32, RC], F32, tag="zd")
                        for kp in range(2):
                            nc.tensor.matmul(
                                psz, w1z[:, 2 * kp:2 * kp + 2, :],
                                xt8[:, 2 * kp:2 * kp + 2, :],
                                start=(kp == 0), stop=(kp == 1), perf_mode=DR)
                        nc.vector.tensor_scalar(out=zT[:, rs], in0=psz,
                                                scalar1=1.0 / WS, scalar2=None,
                                                op0=OP.mult)

                        h2t = h2_pool.tile([128, 4, RC], FP8, tag="h2")
                        for n in range(4):
                            ps = psA.tile([128, RC], F32, tag="mm")
                            for kp in range(4):
                                nc.tensor.matmul(
                                    ps, w2[:, 2 * kp:2 * kp + 2,
                                           128 * n:128 * n + 128],
                                    h1t[:, 2 * kp:2 * kp + 2, :],
                                    start=(kp == 0), stop=(kp == 3),
                                    perf_mode=DR)
                            # psum = 64*(h1@W2); relu(ps + 64*b2) = 64*h2
                            if n % 2 == 0:
                                nc.scalar.activation(out=h2t[:, n, :], in_=ps,
                                                     func=AF.Relu,
                                                     bias=db2_sb[:, n:n + 1])
                            else:
                                nc.vector.tensor_scalar(
                                    out=h2t[:, n, :], in0=ps,
                                    scalar1=db2_sb[:, n:n + 1], scalar2=0.0,
                                    op0=OP.add, op1=OP.max)

                        psd = psZD.tile([32, RC], F32, tag="zd")
                        for kp in range(2):
                            nc.tensor.matmul(
                                psd[0:1, :], w3p[:, 2 * kp:2 * kp + 2, 0:1],
                                h2t[:, 2 * kp:2 * kp + 2, :],
                                start=(kp == 0), stop=(kp == 1), perf_mode=DR)
                        dsc = dsc_pool.tile([1, RC], F32, tag="dsc")
                        # psum = 256*d_raw; d = ps/256 + b3
                        nc.vector.tensor_scalar(out=dsc, in0=psd[0:1, :],
                                                scalar1=1.0 / (WS * WS * WS),
                                                scalar2=db3_sb[0:1, 0:1],
                                                op0=OP.mult, op1=OP.add)
                        nc.sync.dma_start(out=dscr[rs], in_=dsc)

                    def phase_b_group(g, it):
                        """One refinement iteration for batch group g
                        (bh blocks [BHG*g, BHG*(g+1)))."""
                        bh = slice(BHG * g, BHG * (g + 1))
                        # ---- geometry (batch-on-partition, fp32) ----
                        bv = pb.tile([128, BHG, 16, 3], F32, tag="bv")
                        for (e0, e1, cs, ps_) in groups:
                            n = e1 - e0
                            in1 = pos3[:, bh, ps_, :]
                            if in1.shape[2] != n:
                                in1 = in1.to_broadcast([128, BHG, n, 3])
                            nc.vector.tensor_tensor(out=bv[:, :, e0:e1, :],
                                                    in0=pos3[:, bh, cs, :],
                                                    in1=in1, op=OP.subtract)
                        sq = pb.tile([128, BHG, 16, 3], F32, tag="sq")
                        nc.vector.tensor_tensor(out=sq, in0=bv, in1=bv,
                                                op=OP.mult)
                        lensq = pb.tile([128, BHG, 16], F32, tag="lensq")
                        nc.vector.tensor_reduce(out=lensq, in_=sq,
                                                axis=mybir.AxisListType.X,
                                                op=OP.add)
                        dl = pb.tile([128, BHG, 16, 4], BF16, tag="dl")
                        nc.scalar.activation(out=dl[:, :, :, 3], in_=lensq,
                                             func=AF.Sqrt)
                        inv = pb.tile([128, BHG, 16], F32, tag="inv")
                        nc.vector.tensor_scalar(out=inv, in0=dl[:, :, :, 3],
                                                scalar1=EPS, scalar2=None,
                                                op0=OP.add)
                        nc.vector.reciprocal(inv, inv)
                        nc.vector.tensor_tensor(
                            out=dl[:, :, :, 0:3], in0=bv,
                            in1=inv[:, :, :, None].to_broadcast(
                                [128, BHG, 16, 3]),
                            op=OP.mult)

                        # transpose dirlen to [(e,4), b] layout
                        dlT = pb.tile([64, BHG, 128], BF16, tag="dlT")
                        for bi in range(BHG):
                            pst = psTR.tile([128, 128], BF16, tag="tr")
                            nc.tensor.transpose(pst[:64, :], dl[:, bi], id_bf)
                            nc.vector.tensor_copy(out=dlT[:, bi, :],
                                                  in_=pst[:64, :])

                        # ---- bone MLP (transposed layout) ----
                        gs = slice(BG * g, BG * (g + 1))
                        g1 = [g1p.tile([128, BHG, 128], BF16, tag=f"g1_{q}",
                                       name=f"g1_{q}_{g}_{it}")
                              for q in range(4)]
                        for q in range(4):
                            ps = psB.tile([128, RC], F32, tag="bmm")
                            nc.tensor.matmul(
                                ps, blkW1[:, 128 * q:128 * q + 128],
                                dlT, start=True, stop=False)
                            nc.tensor.matmul(
                                ps, id_bf, y1h[q][:, gs],
                                start=False, stop=True)
                            # g1 evictions on DVE (g2's stay on ACT)
                            nc.vector.tensor_scalar(
                                out=g1[q], in0=ps,
                                scalar1=cb1_sb, scalar2=0.0,
                                op0=OP.add, op1=OP.max)
                        g2 = [g2p.tile([128, BHG, 128], BF16, tag=f"g2_{c2}",
                                       name=f"g2_{c2}_{g}_{it}")
                              for c2 in range(8)]
                        for c2 in range(8):
                            q, half = c2 // 2, c2 % 2
                            ps = psB.tile([128, RC], F32, tag="bmm")
                            nc.tensor.matmul(
                                ps, blkW2[:, 128 * half:128 * half + 128],
                                g1[q], start=True, stop=True)
                            nc.scalar.activation(out=g2[c2], in_=ps,
                                                 func=AF.Relu, bias=cb2_sb)
                        pu_sb = pb.tile([3, BHG, 128], BF16, tag="pu_sb")
                        psu = psPU.tile([3, RC], F32, tag="pu")
                        for c2 in range(8):
                            nc.tensor.matmul(psu, w3stk, g2[c2],
                                             start=(c2 == 0), stop=(c2 == 7))
                        # 0.1 * (sum/16 + cb3) = sum*(0.1/16) + 0.1*cb3
                        nc.scalar.activation(out=pu_sb, in_=psu,
                                             func=AF.Identity, bias=cb3s,
                                             scale=0.1 / 16.0)
                        # transpose update back to batch-on-partition layout
                        pu_b = pb.tile([128, BHG, 3], F32, tag="pu_b")
                        for bi in range(BHG):
                            pst = psTR.tile([128, 128], BF16, tag="tr")
                            nc.tensor.transpose(pst[:, :3], pu_sb[:, bi, :],
                                                id3_bf)
                            nc.vector.tensor_copy(out=pu_b[:, bi, :],
                                                  in_=pst[:, :3])
                        # pos3 = (pos3 + pu) * conf
                        nc.vector.tensor_tensor(
                            out=pos3[:, bh], in0=pos3[:, bh],
                            in1=pu_b[:, :, None, :].to_broadcast(
                                [128, BHG, J, 3]),
                            op=OP.add)
                        nc.vector.tensor_tensor(
                            out=pos3[:, bh], in0=pos3[:, bh],
                            in1=conf_b[:, bh, :, None].to_broadcast(
                                [128, BHG, J, 3]),
                            op=OP.mult)

                    def phase_b_init(g):
                        bh = slice(BHG * g, BHG * (g + 1))
                        gs = slice(BG * g, BG * (g + 1))
                        nc.scalar.dma_start(
                            out=p2d_b.rearrange("p bh j c -> p bh (j c)")[:, bh],
                            in_=p2d.rearrange("(bh bl) j c -> bl bh (j c)",
                                              bl=128)[:, bh])
                        nc.scalar.dma_start(
                            out=dep_b[:, bh],
                            in_=dscr.rearrange("(bh bl j) -> bl bh j",
                                               bl=128, j=J)[:, bh])
                        nc.vector.tensor_copy(out=pos3[:, bh, :, 0:2],
                                              in_=p2d_b[:, bh])
                        nc.vector.tensor_copy(out=pos3[:, bh, :, 2],
                                              in_=dep_b[:, bh])
                        nc.scalar.dma_start(
                            out=conf_b[:, bh],
                            in_=conf.rearrange("(bh bl) j -> bl bh j",
                                               bl=128)[:, bh])
                        # avgz[e] = z[parent_e] + z[child_e] (0.5 in w1z)
                        zv = zT.rearrange("p (b j) -> p j b", j=J)[:, :, gs]
                        if stage not in ("A",):
                            for (e0, e1, cs, ps_) in groups:
                                n = e1 - e0
                                in1 = zv[:, ps_, :]
                                if in1.shape[1] != n:
                                    in1 = in1.to_broadcast([32, n, BG])
                                nc.vector.tensor_tensor(
                                    out=avgz[:, e0:e1, gs],
                                    in0=zv[:, cs, :], in1=in1, op=OP.add)
                        # scatter avgz -> y1h_q [(4 bones x 32), b]
                        if stage in ("full",):
                            for e in range(16):
                                q, s_ = e // 4, e % 4
                                nc.scalar.dma_start(
                                    out=y1h[q][32 * s_:32 * s_ + 32, gs],
                                    in_=avgz[:, e, gs])

                    p2d_b = const.tile([128, 8, J, 2], F32, tag="p2db")
                    dep_b = const.tile([128, 8, J], F32, tag="depb")
                    if stage == "noy1h":
                        for q in range(4):
                            nc.vector.memset(y1h[q], 0.0)

                    for g in range(NG):
                        for c in range(CPG * g, CPG * (g + 1)):
                            phase_a_chunk(c)
                        if g == 0:
                            _fill_bone_weights()
                        phase_b_init(g)
                        for it in range(niter):
                            phase_b_group(g, it)
                        # per-group output store overlaps later groups
                        nc.scalar.dma_start(
                            out=out.rearrange("(bh bl) j c -> bl bh j c",
                                              bl=128)[:, BHG * g:BHG * (g + 1)],
                            in_=pos3[:, BHG * g:BHG * (g + 1)])

    nc.compile()
    return nc


def _get_nc():
    stage = _os.environ.get("KITRO_STAGE", "full")
    rep = int(_os.environ.get("KITRO_REPEAT", "1"))
    key = f"nc_{stage}_{rep}"
    if key not in _CACHE:
        _CACHE[key] = _build_nc(stage, rep)
    return _CACHE[key]


def _in_maps(inputs):
    maps = []
    for c in range(NCORE):
        bs = slice(c * BC, (c + 1) * BC)
        m = {
            "features": np.ascontiguousarray(inputs["features"][bs]),
            "poses_2d": np.ascontiguousarray(inputs["poses_2d"][bs]),
            "confidence": np.ascontiguousarray(inputs["confidence"][bs]),
        }
        for k in ("dW1", "db1", "dW2", "db2", "dW3", "db3",
                  "cW1", "cb1", "cW2", "cb2", "cW3", "cb3"):
            m[k] = np.ascontiguousarray(inputs[k])
        maps.append(m)
    return maps


def _run(inputs, **kw):
    nc = _get_nc()
    res = run_bass_kernel_spmd(nc, _in_maps(inputs),
                               core_ids=list(range(NCORE)), **kw)
    full = np.concatenate([res.results[c]["out"] for c in range(NCORE)],
                          axis=0)
    return full.astype(np.float32), res


def kernel(**inputs) -> np.ndarray:
    out, _ = _run(inputs)
    return out


# revision 27
# speedup vs baseline: 16714.8737x; 13982.6861x over previous
"""Trainium2 Bass kernel for nn_KITRO (gnn_message_passing).

Pure data parallel over 8 NeuronCores: batch 8192 -> 1024 per core.
Per core:
  Phase A (depth MLP, transposed dataflow, fp8e4 DoubleRow matmuls /
  fp32 accum; weights pre-scaled x16 so fp8 stays in normal range,
  scales folded into biases and PSUM evictions):
    features f32 --cast DMA--> bf16 DRAM scratch --xbar transpose DMA-->
    XT bf16 --DVE cast--> XT fp8 [512f, rows] in SBUF;
    h1T = relu(W1^T XT + b1) (stored 16*h1, fp8);
    h2T = relu(W2^T h1T + b2) (stored 16*h2, fp8);
    depths = W3^T h2T + b3 -> DRAM scratch.
    Also zT = (0.5*cW1[4:])^T XT (bone-feature projection, averaged later
    in 32-dim z space instead of 512-dim feature space -- exact).
  Phase B (3 bone refinement iterations):
    batch-on-partition geometry (bone vectors / length / direction) on
    DVE/ACT, tiny bone MLP via block-diagonal matmuls in transposed
    layout on PE, per-batch mean over bones fused into the last matmul.
"""

import sys

if "/opt/trn_rl_repo" not in sys.path:
    sys.path.insert(0, "/opt/trn_rl_repo")

import contextlib

import os as _os

import numpy as np

import concourse.bass as bass
import concourse.mybir as mybir
import concourse.tile as tile
from concourse import bacc
from concourse.bass_utils import run_bass_kernel_spmd
from concourse.masks import make_identity

F32 = mybir.dt.float32
BF16 = mybir.dt.bfloat16
FP8 = mybir.dt.float8e4
AF = mybir.ActivationFunctionType
OP = mybir.AluOpType
DR = mybir.MatmulPerfMode.DoubleRow

NCORE = 8
B = 8192
BC = B // NCORE          # 1024 batches per core
J = 25
FD = 512
HD = 1024
ROWS = BC * J            # 25600 rows per core
RC = 512                 # row-chunk
NCH = ROWS // RC         # 50 chunks
EPS = 1e-8
WS = 16.0                # fp8 weight pre-scale

_CACHE = {}


def _build_nc(stage="full", rep=1):
    nc = bacc.Bacc("TRN2", target_bir_lowering=False, debug=False,
                   num_devices=NCORE)

    # ---- per-core DRAM I/O ----
    feats = nc.dram_tensor("features", [BC, J, FD], F32, kind="ExternalInput")
    p2d = nc.dram_tensor("poses_2d", [BC, J, 2], F32, kind="ExternalInput")
    conf = nc.dram_tensor("confidence", [BC, J], F32, kind="ExternalInput")
    dW1 = nc.dram_tensor("dW1", [FD, HD], F32, kind="ExternalInput")
    db1 = nc.dram_tensor("db1", [HD], F32, kind="ExternalInput")
    dW2 = nc.dram_tensor("dW2", [HD, FD], F32, kind="ExternalInput")
    db2 = nc.dram_tensor("db2", [FD], F32, kind="ExternalInput")
    dW3 = nc.dram_tensor("dW3", [FD, 1], F32, kind="ExternalInput")
    db3 = nc.dram_tensor("db3", [1], F32, kind="ExternalInput")
    cW1 = nc.dram_tensor("cW1", [4 + FD, 32], F32, kind="ExternalInput")
    cb1 = nc.dram_tensor("cb1", [32], F32, kind="ExternalInput")
    cW2 = nc.dram_tensor("cW2", [32, 64], F32, kind="ExternalInput")
    cb2 = nc.dram_tensor("cb2", [64], F32, kind="ExternalInput")
    cW3 = nc.dram_tensor("cW3", [64, 3], F32, kind="ExternalInput")
    cb3 = nc.dram_tensor("cb3", [3], F32, kind="ExternalInput")
    out = nc.dram_tensor("out", [BC, J, 3], F32, kind="ExternalOutput")

    (feats, p2d, conf, dW1, db1, dW2, db2, dW3, db3,
     cW1, cb1, cW2, cb2, cW3, cb3, out) = (
        t.ap() for t in (feats, p2d, conf, dW1, db1, dW2, db2, dW3, db3,
                         cW1, cb1, cW2, cb2, cW3, cb3, out))

    feats_flat = feats.flatten_outer_dims()          # [ROWS, FD]

    with tile.TileContext(nc) as tc:
        with contextlib.ExitStack() as ctx:
            const = ctx.enter_context(tc.tile_pool(name="const", bufs=1))
            dram = ctx.enter_context(
                tc.tile_pool(name="dram", bufs=1, space="DRAM"))

            # ---- constants / weights ----
            id_bf = const.tile([128, 128], BF16, tag="id")
            make_identity(nc, id_bf)
            id3_bf = const.tile([3, 3], BF16, tag="id3")
            make_identity(nc, id3_bf)

            # fp8 weights, pre-scaled by WS (=16) via f32 staging + DVE cast
            w1 = const.tile([128, 4, HD], FP8, tag="w1")
            w1z = const.tile([128, 4, 32], FP8, tag="w1z")
            w2 = const.tile([128, 8, FD], FP8, tag="w2")
            w3p = const.tile([128, 4, 16], FP8, tag="w3p")
            # weight staging DMAs ride the Activation HWDGE queue so they
            # don't serialize with the per-chunk transpose loads on SP
            with tc.tile_pool(name="wstage", bufs=1) as wst:
                st1 = wst.tile([128, 4, HD], F32, tag="st1")
                nc.scalar.dma_start(
                    out=st1, in_=dW1.rearrange("(kh p) m -> p kh m", p=128))
                nc.vector.tensor_scalar_mul(w1, st1, WS)
                st2 = wst.tile([128, 8, FD], F32, tag="st2")
                nc.scalar.dma_start(
                    out=st2, in_=dW2.rearrange("(kh p) n -> p kh n", p=128))
                nc.vector.tensor_scalar_mul(w2, st2, WS)
                stz = wst.tile([128, 4, 32], F32, tag="stz")
                nc.scalar.dma_start(
                    out=stz, in_=cW1[4:].rearrange("(kh p) m -> p kh m", p=128))
                # fold the 0.5 bone-average factor into the z projection
                nc.vector.tensor_scalar_mul(w1z, stz, 0.5 * WS)
                st3 = wst.tile([128, 4, 1], F32, tag="st3")
                nc.scalar.dma_start(
                    out=st3, in_=dW3.rearrange("(kh p) o -> p kh o", p=128))
                nc.vector.memset(w3p, 0.0)
                nc.vector.tensor_scalar_mul(w3p[:, :, 0:1], st3, WS)

            # block-diagonal bone weights (bf16, Phase B)
            # block-diagonal bone weight tiles (filled after the Phase A
            # chunk loop is emitted, so their casting SWDGE DMAs queue
            # behind the chunk-0 feature casts on the gpsimd queue)
            blkW1 = const.tile([64, 512], BF16, tag="blkW1")   # 16x cW1[:4]
            blkW2 = const.tile([128, 256], BF16, tag="blkW2")  # 4x cW2
            w3stk = const.tile([128, 3], BF16, tag="w3stk")    # cW3 stacked 2x

            def _fill_bone_weights():
                nc.vector.memset(blkW1, 0.0)
                for d in range(16):
                    nc.gpsimd.dma_start(
                        out=blkW1[4 * d:4 * d + 4, 32 * d:32 * d + 32],
                        in_=cW1[0:4, :])
                nc.vector.memset(blkW2, 0.0)
                for d in range(4):
                    nc.gpsimd.dma_start(
                        out=blkW2[32 * d:32 * d + 32, 64 * d:64 * d + 64],
                        in_=cW2[:, :])
                for d in range(2):
                    nc.gpsimd.dma_start(out=w3stk[64 * d:64 * d + 64, :],
                                        in_=cW3[:, :])

            # biases (depth-MLP biases pre-scaled by WS to match weights)
            db1_sb = const.tile([128, 8], F32, tag="db1")
            nc.sync.dma_start(out=db1_sb,
                              in_=db1.rearrange("(m p) -> p m", p=128))
            nc.vector.tensor_scalar_mul(db1_sb, db1_sb, WS)
            db2_sb = const.tile([128, 4], F32, tag="db2")
            nc.sync.dma_start(out=db2_sb,
                              in_=db2.rearrange("(m p) -> p m", p=128))
            nc.vector.tensor_scalar_mul(db2_sb, db2_sb, WS)
            db3_sb = const.tile([1, 1], F32, tag="db3")
            nc.sync.dma_start(out=db3_sb,
                              in_=db3.rearrange("(a o) -> a o", a=1))
            cb1_sb = const.tile([128, 1], F32, tag="cb1")
            for q in range(4):
                nc.sync.dma_start(out=cb1_sb[32 * q:32 * q + 32, :],
                                  in_=cb1.rearrange("(m o) -> m o", o=1))
            cb2_sb = const.tile([128, 1], F32, tag="cb2")
            for q in range(2):
                nc.sync.dma_start(out=cb2_sb[64 * q:64 * q + 64, :],
                                  in_=cb2.rearrange("(m o) -> m o", o=1))
            cb3s = const.tile([3, 1], F32, tag="cb3")
            nc.sync.dma_start(out=cb3s,
                              in_=cb3.rearrange("(m o) -> m o", o=1))
            nc.vector.tensor_scalar_mul(cb3s, cb3s, 0.1)

            # persistent activations
            zT = const.tile([32, ROWS], BF16, tag="zT")        # [32, (b j)]
            pos3 = const.tile([128, 8, J, 3], F32, tag="pos3")
            conf_b = const.tile([128, 8, J], F32, tag="conf")
            avgz = const.tile([32, 16, BC], BF16, tag="avgz")
            y1h = [const.tile([128, BC], BF16, tag=f"y1h{q}", name=f"y1h{q}")
                   for q in range(4)]
            # full-tile writes so the tile allocator always sees these as
            # written before their partial-partition DMA scatters
            for q in range(4):
                nc.vector.memset(y1h[q], 0.0)

            # DRAM scratch (4 explicitly reused buffers: cast DMA c+4 must
            # wait for transpose c, keeping the SWDGE queue from running
            # 50 chunks ahead and stalling the first transposes)
            xbfs = [dram.tile([RC, FD], BF16, name=f"xbf{i}")
                    for i in range(8)]
            dscr = dram.tile([ROWS], F32)

            # skeleton bone groups: (e0, e1, child slice, parent slice)
            groups = [(0, 6, slice(1, 7), slice(0, 1)),
                      (6, 8, slice(7, 9), slice(5, 7)),
                      (8, 10, slice(9, 11), slice(7, 9)),
                      (10, 12, slice(11, 13), slice(5, 7)),
                      (12, 14, slice(13, 15), slice(11, 13)),
                      (14, 16, slice(15, 17), slice(13, 15))]

            split = _os.environ.get("KITRO_SPLIT", "2way")
            H1_EV = ["a", "v", "a", "v", "a", "v", "a", "v"]
            xt8_pool_cast = split == "3way"
            NG = int(_os.environ.get("KITRO_NG", "2"))  # batch groups (Phase B of group g
            CPG = NCH // NG          # overlaps Phase A of group g+1)
            BHG = 8 // NG            # bh blocks per group
            BG = BC // NG            # batches per group
            niter = 0 if stage in ("A", "B0") else 3

            rep_ctx = tc.For_i(0, rep) if rep > 1 else contextlib.nullcontext()
            with rep_ctx:
                with tc.tile_pool(name="xt", bufs=3) as xt_pool, \
                     tc.tile_pool(name="xt8p", bufs=3) as xt8_pool, \
                     tc.tile_pool(name="h1p", bufs=2) as h1_pool, \
                     tc.tile_pool(name="h2p", bufs=2) as h2_pool, \
                     tc.tile_pool(name="dscp", bufs=3) as dsc_pool, \
                     tc.tile_pool(name="pb", bufs=2) as pb, \
                     tc.tile_pool(name="g1p", bufs=2) as g1p, \
                     tc.tile_pool(name="g2p", bufs=2) as g2p, \
                     tc.tile_pool(name="psA", bufs=4, space="PSUM") as psA, \
                     tc.tile_pool(name="psB", bufs=1, space="PSUM") as psB, \
                     tc.tile_pool(name="psZD", bufs=1, space="PSUM") as psZD, \
                     tc.tile_pool(name="psPU", bufs=1, space="PSUM") as psPU, \
                     tc.tile_pool(name="psTR", bufs=1, space="PSUM") as psTR:

                    def phase_a_chunk(c):
                        rs = slice(c * RC, (c + 1) * RC)
                        # cast fp32 -> bf16 (DRAM->DRAM, SWDGE casts in
                        # flight; 8 rotating buffers bound queue run-ahead)
                        xbf = xbfs[c % 8]
                        nc.gpsimd.dma_start(out=xbf, in_=feats_flat[rs, :])
                        # transposed load via xbar: XT[p, fb, r]
                        xt = xt_pool.tile([128, 4, RC], BF16, tag="xt")
                        nc.sync.dma_start_transpose(xt, xbf)
                        xt8 = xt8_pool.tile([128, 4, RC], FP8, tag="xt8")
                        (nc.gpsimd if xt8_pool_cast else nc.vector).tensor_copy(
                            out=xt8, in_=xt)

                        h1t = h1_pool.tile([128, 8, RC], FP8, tag="h1")
                        for m in range(8):
                            ps = psA.tile([128, RC], F32, tag="mm")
                            for kp in range(2):
                                nc.tensor.matmul(
                                    ps, w1[:, 2 * kp:2 * kp + 2,
                                           128 * m:128 * m + 128],
                                    xt8[:, 2 * kp:2 * kp + 2, :],
                                    start=(kp == 0), stop=(kp == 1),
                                    perf_mode=DR)
                            # psum = 16*(x@W1); out = relu(ps + 16*b1) = 16*h1
                            # split evictions across ACT / DVE / Pool
                            eng = H1_EV[m]
                            if eng == "a":
                                nc.scalar.activation(out=h1t[:, m, :], in_=ps,
                                                     func=AF.Relu,
                                                     bias=db1_sb[:, m:m + 1])
                            else:
                                e = nc.vector if eng == "v" else nc.gpsimd
                                e.tensor_scalar(
                                    out=h1t[:, m, :], in0=ps,
                                    scalar1=db1_sb[:, m:m + 1], scalar2=0.0,
                                    op0=OP.add, op1=OP.max)
                        # z projection (shares XT); psum = 16*z
                        psz = psZD.tile([32, RC], F32, tag="zd")
                        for kp in range(2):
                            nc.tensor.matmul(
                                psz, w1z[:, 2 * kp:2 * kp + 2, :],
                                xt8[:, 2 * kp:2 * kp + 2, :],
                                start=(kp == 0), stop=(kp == 1), perf_mode=DR)
                        nc.vector.tensor_scalar(out=zT[:, rs], in0=psz,
                                                scalar1=1.0 / WS, scalar2=None,
                                                op0=OP.mult)

                        h2t = h2_pool.tile([128, 4, RC], FP8, tag="h2")
                        for n in range(4):
                            ps = psA.tile([128, RC], F32, tag="mm")
                            for kp in range(4):
                                nc.tensor.matmul(
                                    ps, w2[:, 2 * kp:2 * kp + 2,
                                           128 * n:128 * n + 128],
                                    h1t[:, 2 * kp:2 * kp + 2, :],
                                    start=(kp == 0), stop=(kp == 3),
                                    perf_mode=DR)
                            # psum = 256*(h1@W2); relu(ps/16 + 16*b2) = 16*h2
                            nc.scalar.activation(out=h2t[:, n, :], in_=ps,
                                                 func=AF.Relu, scale=1.0 / WS,
                                                 bias=db2_sb[:, n:n + 1])

                        psd = psZD.tile([32, RC], F32, tag="zd")
                        for kp in range(2):
                            nc.tensor.matmul(
                                psd[0:1, :], w3p[:, 2 * kp:2 * kp + 2, 0:1],
                                h2t[:, 2 * kp:2 * kp + 2, :],
                                start=(kp == 0), stop=(kp == 1), perf_mode=DR)
                        dsc = dsc_pool.tile([1, RC], F32, tag="dsc")
                        # psum = 256*d_raw; d = ps/256 + b3
                        nc.vector.tensor_scalar(out=dsc, in0=psd[0:1, :],
                                                scalar1=1.0 / (WS * WS),
                                                scalar2=db3_sb[0:1, 0:1],
                                                op0=OP.mult, op1=OP.add)
                        nc.sync.dma_start(out=dscr[rs], in_=dsc)

                    def phase_b_group(g, it):
                        """One refinement iteration for batch group g
                        (bh blocks [BHG*g, BHG*(g+1)))."""
                        bh = slice(BHG * g, BHG * (g + 1))
                        # ---- geometry (batch-on-partition, fp32) ----
                        bv = pb.tile([128, BHG, 16, 3], F32, tag="bv")
                        for (e0, e1, cs, ps_) in groups:
                            n = e1 - e0
                            in1 = pos3[:, bh, ps_, :]
                            if in1.shape[2] != n:
                                in1 = in1.to_broadcast([128, BHG, n, 3])
                            nc.vector.tensor_tensor(out=bv[:, :, e0:e1, :],
                                                    in0=pos3[:, bh, cs, :],
                                                    in1=in1, op=OP.subtract)
                        sq = pb.tile([128, BHG, 16, 3], F32, tag="sq")
                        nc.vector.tensor_tensor(out=sq, in0=bv, in1=bv,
                                                op=OP.mult)
                        lensq = pb.tile([128, BHG, 16], F32, tag="lensq")
                        nc.vector.tensor_reduce(out=lensq, in_=sq,
                                                axis=mybir.AxisListType.X,
                                                op=OP.add)
                        dl = pb.tile([128, BHG, 16, 4], BF16, tag="dl")
                        nc.scalar.activation(out=dl[:, :, :, 3], in_=lensq,
                                             func=AF.Sqrt)
                        inv = pb.tile([128, BHG, 16], F32, tag="inv")
                        nc.vector.tensor_scalar(out=inv, in0=dl[:, :, :, 3],
                                                scalar1=EPS, scalar2=None,
                                                op0=OP.add)
                        nc.vector.reciprocal(inv, inv)
                        nc.vector.tensor_tensor(
                            out=dl[:, :, :, 0:3], in0=bv,
                            in1=inv[:, :, :, None].to_broadcast(
                                [128, BHG, 16, 3]),
                            op=OP.mult)

                        # transpose dirlen to [(e,4), b] layout
                        dlT = pb.tile([64, BHG, 128], BF16, tag="dlT")
                        for bi in range(BHG):
                            pst = psTR.tile([128, 128], BF16, tag="tr")
                            nc.tensor.transpose(pst[:64, :], dl[:, bi], id_bf)
                            nc.vector.tensor_copy(out=dlT[:, bi, :],
                                                  in_=pst[:64, :])

                        # ---- bone MLP (transposed layout) ----
                        gs = slice(BG * g, BG * (g + 1))
                        g1 = [g1p.tile([128, BHG, 128], BF16, tag=f"g1_{q}",
                                       name=f"g1_{q}_{g}_{it}")
                              for q in range(4)]
                        for q in range(4):
                            ps = psB.tile([128, RC], F32, tag="bmm")
                            nc.tensor.matmul(
                                ps, blkW1[:, 128 * q:128 * q + 128],
                                dlT, start=True, stop=False)
                            nc.tensor.matmul(
                                ps, id_bf, y1h[q][:, gs],
                                start=False, stop=True)
                            # g1 evictions on DVE (g2's stay on ACT)
                            nc.vector.tensor_scalar(
                                out=g1[q], in0=ps,
                                scalar1=cb1_sb, scalar2=0.0,
                                op0=OP.add, op1=OP.max)
                        g2 = [g2p.tile([128, BHG, 128], BF16, tag=f"g2_{c2}",
                                       name=f"g2_{c2}_{g}_{it}")
                              for c2 in range(8)]
                        for c2 in range(8):
                            q, half = c2 // 2, c2 % 2
                            ps = psB.tile([128, RC], F32, tag="bmm")
                            nc.tensor.matmul(
                                ps, blkW2[:, 128 * half:128 * half + 128],
                                g1[q], start=True, stop=True)
                            nc.scalar.activation(out=g2[c2], in_=ps,
                                                 func=AF.Relu, bias=cb2_sb)
                        pu_sb = pb.tile([3, BHG, 128], BF16, tag="pu_sb")
                        psu = psPU.tile([3, RC], F32, tag="pu")
                        for c2 in range(8):
                            nc.tensor.matmul(psu, w3stk, g2[c2],
                                             start=(c2 == 0), stop=(c2 == 7))
                        # 0.1 * (sum/16 + cb3) = sum*(0.1/16) + 0.1*cb3
                        nc.scalar.activation(out=pu_sb, in_=psu,
                                             func=AF.Identity, bias=cb3s,
                                             scale=0.1 / 16.0)
                        # transpose update back to batch-on-partition layout
                        pu_b = pb.tile([128, BHG, 3], F32, tag="pu_b")
                        for bi in range(BHG):
                            pst = psTR.tile([128, 128], BF16, tag="tr")
                            nc.tensor.transpose(pst[:, :3], pu_sb[:, bi, :],
                                                id3_bf)
                            nc.vector.tensor_copy(out=pu_b[:, bi, :],
                                                  in_=pst[:, :3])
                        # pos3 = (pos3 + pu) * conf
                        nc.vector.tensor_tensor(
                            out=pos3[:, bh], in0=pos3[:, bh],
                            in1=pu_b[:, :, None, :].to_broadcast(
                                [128, BHG, J, 3]),
                            op=OP.add)
                        nc.vector.tensor_tensor(
                            out=pos3[:, bh], in0=pos3[:, bh],
                            in1=conf_b[:, bh, :, None].to_broadcast(
                                [128, BHG, J, 3]),
                            op=OP.mult)

                    def phase_b_init(g):
                        bh = slice(BHG * g, BHG * (g + 1))
                        gs = slice(BG * g, BG * (g + 1))
                        nc.scalar.dma_start(
                            out=p2d_b.rearrange("p bh j c -> p bh (j c)")[:, bh],
                            in_=p2d.rearrange("(bh bl) j c -> bl bh (j c)",
                                              bl=128)[:, bh])
                        nc.scalar.dma_start(
                            out=dep_b[:, bh],
                            in_=dscr.rearrange("(bh bl j) -> bl bh j",
                                               bl=128, j=J)[:, bh])
                        nc.vector.tensor_copy(out=pos3[:, bh, :, 0:2],
                                              in_=p2d_b[:, bh])
                        nc.vector.tensor_copy(out=pos3[:, bh, :, 2],
                                              in_=dep_b[:, bh])
                        nc.scalar.dma_start(
                            out=conf_b[:, bh],
                            in_=conf.rearrange("(bh bl) j -> bl bh j",
                                               bl=128)[:, bh])
                        # avgz[e] = z[parent_e] + z[child_e] (0.5 in w1z)
                        zv = zT.rearrange("p (b j) -> p j b", j=J)[:, :, gs]
                        if stage not in ("A",):
                            for (e0, e1, cs, ps_) in groups:
                                n = e1 - e0
                                in1 = zv[:, ps_, :]
                                if in1.shape[1] != n:
                                    in1 = in1.to_broadcast([32, n, BG])
                                nc.vector.tensor_tensor(
                                    out=avgz[:, e0:e1, gs],
                                    in0=zv[:, cs, :], in1=in1, op=OP.add)
                        # scatter avgz -> y1h_q [(4 bones x 32), b]
                        if stage in ("full",):
                            for e in range(16):
                                q, s_ = e // 4, e % 4
                                nc.scalar.dma_start(
                                    out=y1h[q][32 * s_:32 * s_ + 32, gs],
                                    in_=avgz[:, e, gs])

                    p2d_b = const.tile([128, 8, J, 2], F32, tag="p2db")
                    dep_b = const.tile([128, 8, J], F32, tag="depb")
                    if stage == "noy1h":
                        for q in range(4):
                            nc.vector.memset(y1h[q], 0.0)

                    def phase_b_all(g):
                        if g == 0:
                            _fill_bone_weights()
                        phase_b_init(g)
                        for it in range(niter):
                            phase_b_group(g, it)
                        # per-group output store overlaps later groups
                        nc.scalar.dma_start(
                            out=out.rearrange("(bh bl) j c -> bl bh j c",
                                              bl=128)[:, BHG * g:BHG * (g + 1)],
                            in_=pos3[:, BHG * g:BHG * (g + 1)])

                    for g in range(NG):
                        for c in range(CPG * g, CPG * (g + 1)):
                            phase_a_chunk(c)
                        phase_b_all(g)

    nc.compile()
    return nc


def _get_nc():
    stage = _os.environ.get("KITRO_STAGE", "full")
    rep = int(_os.environ.get("KITRO_REPEAT", "1"))
    key = f"nc_{stage}_{rep}"
    if key not in _CACHE:
        _CACHE[key] = _build_nc(stage, rep)
    return _CACHE[key]


def _in_maps(inputs):
    maps = []
    for c in range(NCORE):
        bs = slice(c * BC, (c + 1) * BC)
        m = {
            "features": np.ascontiguousarray(inputs["features"][bs]),
            "poses_2d": np.ascontiguousarray(inputs["poses_2d"][bs]),
            "confidence": np.ascontiguousarray(inputs["confidence"][bs]),
        }
        for k in ("dW1", "db1", "dW2", "db2", "dW3", "db3",
                  "cW1", "cb1", "cW2", "cb2", "cW3", "cb3"):
            m[k] = np.ascontiguousarray(inputs[k])
        maps.append(m)
    return maps


def _run(inputs, **kw):
    nc = _get_nc()
    res = run_bass_kernel_spmd(nc, _in_maps(inputs),
                               core_ids=list(range(NCORE)), **kw)
    full = np.concatenate([res.results[c]["out"] for c in range(NCORE)],
                          axis=0)
    return full.astype(np.float32), res


def kernel(**inputs) -> np.ndarray:
    out, _ = _run(inputs)
    return out


# revision 29
# speedup vs baseline: 19982.3429x; 1.1955x over previous
"""Trainium2 Bass kernel for nn_KITRO (gnn_message_passing).

Pure data parallel over 8 NeuronCores: batch 8192 -> 1024 per core.
Per core:
  Phase A (depth MLP, transposed dataflow, fp8e4 DoubleRow matmuls /
  fp32 accum; weights pre-scaled x16 so fp8 stays in normal range,
  scales folded into biases and PSUM evictions):
    features f32 --cast DMA--> bf16 DRAM scratch --xbar transpose DMA-->
    XT bf16 --DVE cast--> XT fp8 [512f, rows] in SBUF;
    h1T = relu(W1^T XT + b1) (stored 16*h1, fp8);
    h2T = relu(W2^T h1T + b2) (stored 16*h2, fp8);
    depths = W3^T h2T + b3 -> DRAM scratch.
    Also zT = (0.5*cW1[4:])^T XT (bone-feature projection, averaged later
    in 32-dim z space instead of 512-dim feature space -- exact).
  Phase B (3 bone refinement iterations):
    batch-on-partition geometry (bone vectors / length / direction) on
    DVE/ACT, tiny bone MLP via block-diagonal matmuls in transposed
    layout on PE, per-batch mean over bones fused into the last matmul.
"""

import sys

if "/opt/trn_rl_repo" not in sys.path:
    sys.path.insert(0, "/opt/trn_rl_repo")

import contextlib

import os as _os

import numpy as np

import concourse.bass as bass
import concourse.mybir as mybir
import concourse.tile as tile
from concourse import bacc
from concourse.bass_utils import run_bass_kernel_spmd
from concourse.masks import make_identity

F32 = mybir.dt.float32
BF16 = mybir.dt.bfloat16
FP8 = mybir.dt.float8e4
AF = mybir.ActivationFunctionType
OP = mybir.AluOpType
DR = mybir.MatmulPerfMode.DoubleRow

NCORE = 8
B = 8192
BC = B // NCORE          # 1024 batches per core
J = 25
FD = 512
HD = 1024
ROWS = BC * J            # 25600 rows per core
RC = 512                 # row-chunk
NCH = ROWS // RC         # 50 chunks
EPS = 1e-8
WS = 16.0                # fp8 weight pre-scale

_CACHE = {}


def _build_nc(stage="full", rep=1):
    nc = bacc.Bacc("TRN2", target_bir_lowering=False, debug=False,
                   num_devices=NCORE)

    # ---- per-core DRAM I/O ----
    feats = nc.dram_tensor("features", [BC, J, FD], F32, kind="ExternalInput")
    p2d = nc.dram_tensor("poses_2d", [BC, J, 2], F32, kind="ExternalInput")
    conf = nc.dram_tensor("confidence", [BC, J], F32, kind="ExternalInput")
    dW1 = nc.dram_tensor("dW1", [FD, HD], F32, kind="ExternalInput")
    db1 = nc.dram_tensor("db1", [HD], F32, kind="ExternalInput")
    dW2 = nc.dram_tensor("dW2", [HD, FD], F32, kind="ExternalInput")
    db2 = nc.dram_tensor("db2", [FD], F32, kind="ExternalInput")
    dW3 = nc.dram_tensor("dW3", [FD, 1], F32, kind="ExternalInput")
    db3 = nc.dram_tensor("db3", [1], F32, kind="ExternalInput")
    cW1 = nc.dram_tensor("cW1", [4 + FD, 32], F32, kind="ExternalInput")
    cb1 = nc.dram_tensor("cb1", [32], F32, kind="ExternalInput")
    cW2 = nc.dram_tensor("cW2", [32, 64], F32, kind="ExternalInput")
    cb2 = nc.dram_tensor("cb2", [64], F32, kind="ExternalInput")
    cW3 = nc.dram_tensor("cW3", [64, 3], F32, kind="ExternalInput")
    cb3 = nc.dram_tensor("cb3", [3], F32, kind="ExternalInput")
    out = nc.dram_tensor("out", [BC, J, 3], F32, kind="ExternalOutput")

    (feats, p2d, conf, dW1, db1, dW2, db2, dW3, db3,
     cW1, cb1, cW2, cb2, cW3, cb3, out) = (
        t.ap() for t in (feats, p2d, conf, dW1, db1, dW2, db2, dW3, db3,
                         cW1, cb1, cW2, cb2, cW3, cb3, out))

    feats_flat = feats.flatten_outer_dims()          # [ROWS, FD]

    with tile.TileContext(nc) as tc:
        with contextlib.ExitStack() as ctx:
            const = ctx.enter_context(tc.tile_pool(name="const", bufs=1))
            dram = ctx.enter_context(
                tc.tile_pool(name="dram", bufs=1, space="DRAM"))

            # ---- constants / weights ----
            id_bf = const.tile([128, 128], BF16, tag="id")
            make_identity(nc, id_bf)
            id3_bf = const.tile([3, 3], BF16, tag="id3")
            make_identity(nc, id3_bf)

            # fp8 weights, pre-scaled by WS (=16) via f32 staging + DVE cast
            w1 = const.tile([128, 4, HD], FP8, tag="w1")
            w1z = const.tile([128, 4, 32], FP8, tag="w1z")
            w2 = const.tile([128, 8, FD], FP8, tag="w2")
            w3p = const.tile([128, 4, 16], FP8, tag="w3p")
            # weight staging DMAs ride the Activation HWDGE queue so they
            # don't serialize with the per-chunk transpose loads on SP
            with tc.tile_pool(name="wstage", bufs=1) as wst:
                st1 = wst.tile([128, 4, HD], F32, tag="st1")
                nc.scalar.dma_start(
                    out=st1, in_=dW1.rearrange("(kh p) m -> p kh m", p=128))
                nc.vector.tensor_scalar_mul(w1, st1, WS)
                st2 = wst.tile([128, 8, FD], F32, tag="st2")
                nc.scalar.dma_start(
                    out=st2, in_=dW2.rearrange("(kh p) n -> p kh n", p=128))
                nc.vector.tensor_scalar_mul(w2, st2, WS)
                stz = wst.tile([128, 4, 32], F32, tag="stz")
                nc.scalar.dma_start(
                    out=stz, in_=cW1[4:].rearrange("(kh p) m -> p kh m", p=128))
                # fold the 0.5 bone-average factor into the z projection
                nc.vector.tensor_scalar_mul(w1z, stz, 0.5 * WS)
                st3 = wst.tile([128, 4, 1], F32, tag="st3")
                nc.scalar.dma_start(
                    out=st3, in_=dW3.rearrange("(kh p) o -> p kh o", p=128))
                nc.vector.memset(w3p, 0.0)
                nc.vector.tensor_scalar_mul(w3p[:, :, 0:1], st3, WS)

            # block-diagonal bone weights (bf16, Phase B)
            # block-diagonal bone weight tiles (filled after the Phase A
            # chunk loop is emitted, so their casting SWDGE DMAs queue
            # behind the chunk-0 feature casts on the gpsimd queue)
            blkW1 = const.tile([64, 512], BF16, tag="blkW1")   # 16x cW1[:4]
            blkW2 = const.tile([128, 256], BF16, tag="blkW2")  # 4x cW2
            w3stk = const.tile([128, 3], BF16, tag="w3stk")    # cW3 stacked 2x

            def _fill_bone_weights():
                nc.vector.memset(blkW1, 0.0)
                for d in range(16):
                    nc.gpsimd.dma_start(
                        out=blkW1[4 * d:4 * d + 4, 32 * d:32 * d + 32],
                        in_=cW1[0:4, :])
                nc.vector.memset(blkW2, 0.0)
                for d in range(4):
                    nc.gpsimd.dma_start(
                        out=blkW2[32 * d:32 * d + 32, 64 * d:64 * d + 64],
                        in_=cW2[:, :])
                for d in range(2):
                    nc.gpsimd.dma_start(out=w3stk[64 * d:64 * d + 64, :],
                                        in_=cW3[:, :])

            # biases (depth-MLP biases pre-scaled by WS to match weights)
            db1_sb = const.tile([128, 8], F32, tag="db1")
            nc.sync.dma_start(out=db1_sb,
                              in_=db1.rearrange("(m p) -> p m", p=128))
            nc.vector.tensor_scalar_mul(db1_sb, db1_sb, WS)
            db2_sb = const.tile([128, 4], F32, tag="db2")
            nc.sync.dma_start(out=db2_sb,
                              in_=db2.rearrange("(m p) -> p m", p=128))
            nc.vector.tensor_scalar_mul(db2_sb, db2_sb, WS)
            db3_sb = const.tile([1, 1], F32, tag="db3")
            nc.sync.dma_start(out=db3_sb,
                              in_=db3.rearrange("(a o) -> a o", a=1))
            cb1_sb = const.tile([128, 1], F32, tag="cb1")
            for q in range(4):
                nc.sync.dma_start(out=cb1_sb[32 * q:32 * q + 32, :],
                                  in_=cb1.rearrange("(m o) -> m o", o=1))
            cb2_sb = const.tile([128, 1], F32, tag="cb2")
            for q in range(2):
                nc.sync.dma_start(out=cb2_sb[64 * q:64 * q + 64, :],
                                  in_=cb2.rearrange("(m o) -> m o", o=1))
            cb3s = const.tile([3, 1], F32, tag="cb3")
            nc.sync.dma_start(out=cb3s,
                              in_=cb3.rearrange("(m o) -> m o", o=1))
            nc.vector.tensor_scalar_mul(cb3s, cb3s, 0.1)

            # persistent activations
            zT = const.tile([32, ROWS], BF16, tag="zT")        # [32, (b j)]
            pos3 = const.tile([128, 8, J, 3], F32, tag="pos3")
            conf_b = const.tile([128, 8, J], F32, tag="conf")
            avgz = const.tile([32, 16, BC], BF16, tag="avgz")
            y1h = [const.tile([128, BC], BF16, tag=f"y1h{q}", name=f"y1h{q}")
                   for q in range(4)]
            # full-tile writes so the tile allocator always sees these as
            # written before their partial-partition DMA scatters
            for q in range(4):
                nc.vector.memset(y1h[q], 0.0)

            # DRAM scratch (4 explicitly reused buffers: cast DMA c+4 must
            # wait for transpose c, keeping the SWDGE queue from running
            # 50 chunks ahead and stalling the first transposes)
            xbfs = [dram.tile([RC, FD], BF16, name=f"xbf{i}")
                    for i in range(8)]
            dscr = dram.tile([ROWS], F32)

            # skeleton bone groups: (e0, e1, child slice, parent slice)
            groups = [(0, 6, slice(1, 7), slice(0, 1)),
                      (6, 8, slice(7, 9), slice(5, 7)),
                      (8, 10, slice(9, 11), slice(7, 9)),
                      (10, 12, slice(11, 13), slice(5, 7)),
                      (12, 14, slice(13, 15), slice(11, 13)),
                      (14, 16, slice(15, 17), slice(13, 15))]

            split = _os.environ.get("KITRO_SPLIT", "2way")
            H1_EV = ["a", "v", "a", "v", "a", "v", "a", "v"]
            xt8_pool_cast = split == "3way"
            NG = int(_os.environ.get("KITRO_NG", "2"))  # batch groups (Phase B of group g
            CPG = NCH // NG          # overlaps Phase A of group g+1)
            BHG = 8 // NG            # bh blocks per group
            BG = BC // NG            # batches per group
            niter = 0 if stage in ("A", "B0") else 3

            rep_ctx = tc.For_i(0, rep) if rep > 1 else contextlib.nullcontext()
            with rep_ctx:
                with tc.tile_pool(name="xt", bufs=3) as xt_pool, \
                     tc.tile_pool(name="xt8p", bufs=3) as xt8_pool, \
                     tc.tile_pool(name="h1p", bufs=2) as h1_pool, \
                     tc.tile_pool(name="h2p", bufs=2) as h2_pool, \
                     tc.tile_pool(name="dscp", bufs=3) as dsc_pool, \
                     tc.tile_pool(name="pb", bufs=2) as pb, \
                     tc.tile_pool(name="g1p", bufs=2) as g1p, \
                     tc.tile_pool(name="g2p", bufs=2) as g2p, \
                     tc.tile_pool(name="psA", bufs=4, space="PSUM") as psA, \
                     tc.tile_pool(name="psB", bufs=1, space="PSUM") as psB, \
                     tc.tile_pool(name="psZD", bufs=1, space="PSUM") as psZD, \
                     tc.tile_pool(name="psPU", bufs=1, space="PSUM") as psPU, \
                     tc.tile_pool(name="psTR", bufs=1, space="PSUM") as psTR:

                    def phase_a_chunk(c):
                        rs = slice(c * RC, (c + 1) * RC)
                        # cast fp32 -> bf16 (DRAM->DRAM, SWDGE casts in
                        # flight; 8 rotating buffers bound queue run-ahead)
                        xbf = xbfs[c % 8]
                        nc.gpsimd.dma_start(out=xbf, in_=feats_flat[rs, :])
                        # transposed load via xbar: XT[p, fb, r]
                        xt = xt_pool.tile([128, 4, RC], BF16, tag="xt")
                        nc.sync.dma_start_transpose(xt, xbf)
                        xt8 = xt8_pool.tile([128, 4, RC], FP8, tag="xt8")
                        (nc.gpsimd if xt8_pool_cast else nc.vector).tensor_copy(
                            out=xt8, in_=xt)

                        h1t = h1_pool.tile([128, 8, RC], FP8, tag="h1")
                        for m in range(8):
                            ps = psA.tile([128, RC], F32, tag="mm")
                            for kp in range(2):
                                nc.tensor.matmul(
                                    ps, w1[:, 2 * kp:2 * kp + 2,
                                           128 * m:128 * m + 128],
                                    xt8[:, 2 * kp:2 * kp + 2, :],
                                    start=(kp == 0), stop=(kp == 1),
                                    perf_mode=DR)
                            # psum = 16*(x@W1); out = relu(ps + 16*b1) = 16*h1
                            # split evictions across ACT / DVE / Pool
                            eng = H1_EV[m]
                            if eng == "a":
                                nc.scalar.activation(out=h1t[:, m, :], in_=ps,
                                                     func=AF.Relu,
                                                     bias=db1_sb[:, m:m + 1])
                            else:
                                e = nc.vector if eng == "v" else nc.gpsimd
                                e.tensor_scalar(
                                    out=h1t[:, m, :], in0=ps,
                                    scalar1=db1_sb[:, m:m + 1], scalar2=0.0,
                                    op0=OP.add, op1=OP.max)
                        # z projection (shares XT); psum = 16*z
                        psz = psZD.tile([32, RC], F32, tag="zd")
                        for kp in range(2):
                            nc.tensor.matmul(
                                psz, w1z[:, 2 * kp:2 * kp + 2, :],
                                xt8[:, 2 * kp:2 * kp + 2, :],
                                start=(kp == 0), stop=(kp == 1), perf_mode=DR)
                        nc.vector.tensor_scalar(out=zT[:, rs], in0=psz,
                                                scalar1=1.0 / WS, scalar2=None,
                                                op0=OP.mult)

                        h2t = h2_pool.tile([128, 4, RC], FP8, tag="h2")
                        for n in range(4):
                            ps = psA.tile([128, RC], F32, tag="mm")
                            for kp in range(4):
                                nc.tensor.matmul(
                                    ps, w2[:, 2 * kp:2 * kp + 2,
                                           128 * n:128 * n + 128],
                                    h1t[:, 2 * kp:2 * kp + 2, :],
                                    start=(kp == 0), stop=(kp == 3),
                                    perf_mode=DR)
                            # psum = 256*(h1@W2); relu(ps/16 + 16*b2) = 16*h2
                            nc.scalar.activation(out=h2t[:, n, :], in_=ps,
                                                 func=AF.Relu, scale=1.0 / WS,
                                                 bias=db2_sb[:, n:n + 1])

                        psd = psZD.tile([32, RC], F32, tag="zd")
                        for kp in range(2):
                            nc.tensor.matmul(
                                psd[0:1, :], w3p[:, 2 * kp:2 * kp + 2, 0:1],
                                h2t[:, 2 * kp:2 * kp + 2, :],
                                start=(kp == 0), stop=(kp == 1), perf_mode=DR)
                        dsc = dsc_pool.tile([1, RC], F32, tag="dsc")
                        # psum = 256*d_raw; d = ps/256 + b3
                        nc.vector.tensor_scalar(out=dsc, in0=psd[0:1, :],
                                                scalar1=1.0 / (WS * WS),
                                                scalar2=db3_sb[0:1, 0:1],
                                                op0=OP.mult, op1=OP.add)
                        nc.sync.dma_start(out=dscr[rs], in_=dsc)

                    def phase_b_group(g, it):
                        """One refinement iteration for batch group g
                        (bh blocks [BHG*g, BHG*(g+1)))."""
                        bh = slice(BHG * g, BHG * (g + 1))
                        # ---- geometry (batch-on-partition, fp32) ----
                        bv = pb.tile([128, BHG, 16, 3], F32, tag="bv")
                        for (e0, e1, cs, ps_) in groups:
                            n = e1 - e0
                            in1 = pos3[:, bh, ps_, :]
                            if in1.shape[2] != n:
                                in1 = in1.to_broadcast([128, BHG, n, 3])
                            nc.vector.tensor_tensor(out=bv[:, :, e0:e1, :],
                                                    in0=pos3[:, bh, cs, :],
                                                    in1=in1, op=OP.subtract)
                        sq = pb.tile([128, BHG, 16, 3], F32, tag="sq")
                        nc.vector.tensor_tensor(out=sq, in0=bv, in1=bv,
                                                op=OP.mult)
                        lensq = pb.tile([128, BHG, 16], F32, tag="lensq")
                        nc.vector.tensor_reduce(out=lensq, in_=sq,
                                                axis=mybir.AxisListType.X,
                                                op=OP.add)
                        dl = pb.tile([128, BHG, 16, 4], BF16, tag="dl")
                        nc.scalar.activation(out=dl[:, :, :, 3], in_=lensq,
                                             func=AF.Sqrt)
                        inv = pb.tile([128, BHG, 16], F32, tag="inv")
                        nc.vector.tensor_scalar(out=inv, in0=dl[:, :, :, 3],
                                                scalar1=EPS, scalar2=None,
                                                op0=OP.add)
                        nc.vector.reciprocal(inv, inv)
                        nc.vector.tensor_tensor(
                            out=dl[:, :, :, 0:3], in0=bv,
                            in1=inv[:, :, :, None].to_broadcast(
                                [128, BHG, 16, 3]),
                            op=OP.mult)

                        # transpose dirlen to [(e,4), b] layout
                        dlT = pb.tile([64, BHG, 128], BF16, tag="dlT")
                        for bi in range(BHG):
                            pst = psTR.tile([128, 128], BF16, tag="tr")
                            nc.tensor.transpose(pst[:64, :], dl[:, bi], id_bf)
                            nc.vector.tensor_copy(out=dlT[:, bi, :],
                                                  in_=pst[:64, :])

                        # ---- bone MLP (transposed layout) ----
                        gs = slice(BG * g, BG * (g + 1))
                        g1 = [g1p.tile([128, BHG, 128], BF16, tag=f"g1_{q}",
                                       name=f"g1_{q}_{g}_{it}")
                              for q in range(4)]
                        for q in range(4):
                            ps = psB.tile([128, RC], F32, tag="bmm")
                            nc.tensor.matmul(
                                ps, blkW1[:, 128 * q:128 * q + 128],
                                dlT, start=True, stop=False)
                            nc.tensor.matmul(
                                ps, id_bf, y1h[q][:, gs],
                                start=False, stop=True)
                            # g1 evictions on DVE (g2's stay on ACT)
                            nc.vector.tensor_scalar(
                                out=g1[q], in0=ps,
                                scalar1=cb1_sb, scalar2=0.0,
                                op0=OP.add, op1=OP.max)
                        g2 = [g2p.tile([128, BHG, 128], BF16, tag=f"g2_{c2}",
                                       name=f"g2_{c2}_{g}_{it}")
                              for c2 in range(8)]
                        for c2 in range(8):
                            q, half = c2 // 2, c2 % 2
                            ps = psB.tile([128, RC], F32, tag="bmm")
                            nc.tensor.matmul(
                                ps, blkW2[:, 128 * half:128 * half + 128],
                                g1[q], start=True, stop=True)
                            nc.scalar.activation(out=g2[c2], in_=ps,
                                                 func=AF.Relu, bias=cb2_sb)
                        pu_sb = pb.tile([3, BHG, 128], BF16, tag="pu_sb")
                        psu = psPU.tile([3, RC], F32, tag="pu")
                        for c2 in range(8):
                            nc.tensor.matmul(psu, w3stk, g2[c2],
                                             start=(c2 == 0), stop=(c2 == 7))
                        # 0.1 * (sum/16 + cb3) = sum*(0.1/16) + 0.1*cb3
                        nc.scalar.activation(out=pu_sb, in_=psu,
                                             func=AF.Identity, bias=cb3s,
                                             scale=0.1 / 16.0)
                        # transpose update back to batch-on-partition layout
                        pu_b = pb.tile([128, BHG, 3], F32, tag="pu_b")
                        for bi in range(BHG):
                            pst = psTR.tile([128, 128], BF16, tag="tr")
                            nc.tensor.transpose(pst[:, :3], pu_sb[:, bi, :],
                                                id3_bf)
                            nc.vector.tensor_copy(out=pu_b[:, bi, :],
                                                  in_=pst[:, :3])
                        # pos3 = (pos3 + pu) * conf
                        nc.vector.tensor_tensor(
                            out=pos3[:, bh], in0=pos3[:, bh],
                            in1=pu_b[:, :, None, :].to_broadcast(
                                [128, BHG, J, 3]),
                            op=OP.add)
                        nc.vector.tensor_tensor(
                            out=pos3[:, bh], in0=pos3[:, bh],
                            in1=conf_b[:, bh, :, None].to_broadcast(
                                [128, BHG, J, 3]),
                            op=OP.mult)

                    def phase_b_init(g):
                        bh = slice(BHG * g, BHG * (g + 1))
                        gs = slice(BG * g, BG * (g + 1))
                        nc.scalar.dma_start(
                            out=p2d_b.rearrange("p bh j c -> p bh (j c)")[:, bh],
                            in_=p2d.rearrange("(bh bl) j c -> bl bh (j c)",
                                              bl=128)[:, bh])
                        nc.scalar.dma_start(
                            out=dep_b[:, bh],
                            in_=dscr.rearrange("(bh bl j) -> bl bh j",
                                               bl=128, j=J)[:, bh])
                        nc.vector.tensor_copy(out=pos3[:, bh, :, 0:2],
                                              in_=p2d_b[:, bh])
                        nc.vector.tensor_copy(out=pos3[:, bh, :, 2],
                                              in_=dep_b[:, bh])
                        nc.scalar.dma_start(
                            out=conf_b[:, bh],
                            in_=conf.rearrange("(bh bl) j -> bl bh j",
                                               bl=128)[:, bh])
                        # avgz[e] = z[parent_e] + z[child_e] (0.5 in w1z)
                        zv = zT.rearrange("p (b j) -> p j b", j=J)[:, :, gs]
                        if stage not in ("A",):
                            for (e0, e1, cs, ps_) in groups:
                                n = e1 - e0
                                in1 = zv[:, ps_, :]
                                if in1.shape[1] != n:
                                    in1 = in1.to_broadcast([32, n, BG])
                                nc.vector.tensor_tensor(
                                    out=avgz[:, e0:e1, gs],
                                    in0=zv[:, cs, :], in1=in1, op=OP.add)
                        # scatter avgz -> y1h_q [(4 bones x 32), b]
                        if stage in ("full",):
                            for e in range(16):
                                q, s_ = e // 4, e % 4
                                nc.scalar.dma_start(
                                    out=y1h[q][32 * s_:32 * s_ + 32, gs],
                                    in_=avgz[:, e, gs])

                    p2d_b = const.tile([128, 8, J, 2], F32, tag="p2db")
                    dep_b = const.tile([128, 8, J], F32, tag="depb")
                    if stage == "noy1h":
                        for q in range(4):
                            nc.vector.memset(y1h[q], 0.0)

                    def phase_b_all(g):
                        if g == 0:
                            _fill_bone_weights()
                        phase_b_init(g)
                        for it in range(niter):
                            phase_b_group(g, it)
                        # per-group output store overlaps later groups
                        nc.scalar.dma_start(
                            out=out.rearrange("(bh bl) j c -> bl bh j c",
                                              bl=128)[:, BHG * g:BHG * (g + 1)],
                            in_=pos3[:, BHG * g:BHG * (g + 1)])

                    for g in range(NG):
                        for c in range(CPG * g, CPG * (g + 1)):
                            phase_a_chunk(c)
                        phase_b_all(g)

    nc.compile()
    return nc


def _get_nc():
    stage = _os.environ.get("KITRO_STAGE", "full")
    rep = int(_os.environ.get("KITRO_REPEAT", "1"))
    key = f"nc_{stage}_{rep}"
    if key not in _CACHE:
        _CACHE[key] = _build_nc(stage, rep)
    return _CACHE[key]


def _in_maps(inputs):
    maps = []
    for c in range(NCORE):
        bs = slice(c * BC, (c + 1) * BC)
        m = {
            "features": np.ascontiguousarray(inputs["features"][bs]),
            "poses_2d": np.ascontiguousarray(inputs["poses_2d"][bs]),
            "confidence": np.ascontiguousarray(inputs["confidence"][bs]),
        }
        for k in ("dW1", "db1", "dW2", "db2", "dW3", "db3",
                  "cW1", "cb1", "cW2", "cb2", "cW3", "cb3"):
            m[k] = np.ascontiguousarray(inputs[k])
        maps.append(m)
    return maps


def _run(inputs, **kw):
    nc = _get_nc()
    res = run_bass_kernel_spmd(nc, _in_maps(inputs),
                               core_ids=list(range(NCORE)), **kw)
    full = np.concatenate([res.results[c]["out"] for c in range(NCORE)],
                          axis=0)
    return full.astype(np.float32), res


def kernel(**inputs) -> np.ndarray:
    out, _ = _run(inputs)
    return out
